# revision 14
# baseline (speedup 1.0000x reference)
"""Cross-attention Trainium2 kernel, 8-core data-parallel.

Problem (hardcoded): B=4, NQ=4096, NK=1024, QD=1024, CD=768, H=16, HD=64.
  out = softmax((x@Wq) @ (ctx@Wk)^T / sqrt(HD)) @ (ctx@Wv) @ Wo + bo

Sharding: pure data-parallel. 8 cores = 4 batches x 2 NQ-halves of 2048
query rows. Each core redundantly computes K/V projections for its batch
(3.2+3.2 GF vs 18.8 GF total per core -- cheap) and needs no collectives.

Per-core layout strategy: every matmul keeps its contraction dim on SBUF
partitions, so the chain is computed fully "transposed":
  QT[qd,q]   = Wq^T-contract(xT)        (x pre-transposed on host)
  KT[qd,k]   = Wk^T-contract(ctxT)
  V'[k,qd+1] = ctx@Wv with a ones column appended per head
  ST[k,q]    = per head: KT_h^T-slices @ QT_h (row-tiled 2 heads/matmul)
  ET         = exp(ST * 1/8)            (ACT, scale folded in)
  O'T[65,q]  = V'_h^T @ ET_h            (row 64 = softmax denominators)
  attnT      = O'T[0:64] * recip(denoms) (DVE, recip broadcast via PE)
  out[q,od]  = attnT^T @ Wo + bo
"""

import numpy as np

B, NQ, NK = 4, 4096, 1024
QD, CD, H = 1024, 768, 16
HD = QD // H
SCALE = HD ** -0.5
NQL = NQ // 2          # query rows per core
N_CORES = 8


def build_bass():
    """Build the per-core Bass graph (SPMD, identical on all 8 cores)."""
    import concourse.bass as bass
    import concourse.tile as tile
    from concourse import bacc, mybir

    f32 = mybir.dt.float32
    bf16 = mybir.dt.bfloat16
    EXP = mybir.ActivationFunctionType.Exp

    nc = bacc.Bacc()

    xT_h = nc.dram_tensor("xT", (QD, NQL), bf16, kind="ExternalInput")
    ctxT_h = nc.dram_tensor("ctxT", (CD, NK), bf16, kind="ExternalInput")
    wq_h = nc.dram_tensor("wq", (QD, QD), bf16, kind="ExternalInput")
    wk_h = nc.dram_tensor("wk", (CD, QD), bf16, kind="ExternalInput")
    wv_h = nc.dram_tensor("wv", (CD, QD), bf16, kind="ExternalInput")
    wo_h = nc.dram_tensor("wo", (QD, QD), bf16, kind="ExternalInput")
    bo_h = nc.dram_tensor("bo", (1, QD), bf16, kind="ExternalInput")
    out_h = nc.dram_tensor("out", (NQL, QD), f32, kind="ExternalOutput")

    # DRAM views chunked to 128 partitions
    xT_d = xT_h[:].rearrange("(c p) n -> p c n", p=128)       # [128, 8, 2048]
    ctxT_d = ctxT_h[:].rearrange("(c p) n -> p c n", p=128)   # [128, 6, 1024]
    wq_d = wq_h[:].rearrange("(c p) m -> p c m", p=128)       # [128, 8, 1024]
    wk_d = wk_h[:].rearrange("(c p) m -> p c m", p=128)       # [128, 6, 1024]
    wv_d = wv_h[:].rearrange("(c p) m -> p c m", p=128)       # [128, 6, 1024]
    wo_d = wo_h[:].rearrange("(c p) m -> p c m", p=128)       # [128, 8, 1024]
    out_d = out_h[:].rearrange("(t p) n -> p t n", p=128)     # [128, 16, 1024]

    KC_Q = QD // 128   # 8  contraction chunks for Q/out projections
    KC_C = CD // 128   # 6  contraction chunks for K/V projections
    NKC = NK // 128    # 8  key chunks
    NQT = NQL // 512   # 4  query tiles of 512
    HP = H // 2        # 8  head pairs

    with tile.TileContext(nc) as tc:
        _pp_cm = tc.tile_pool(name="persist", bufs=1)
        pp = _pp_cm.__enter__()
        # ---- persistent tiles (live for the whole kernel)
        qt_sb = pp.tile([128, KC_Q, NQL], bf16)        # QT   4 MB
        kt_sb = pp.tile([128, KC_Q, NK], bf16)         # KT   2 MB
        vp_sb = pp.tile([128, NKC, H, HD + 1], bf16)   # V'   2.08 MB
        attnT_sb = pp.tile([128, KC_Q, NQL], bf16)     # attn 4 MB
        wo_sb = pp.tile([128, KC_Q, QD], bf16)         # Wo   2 MB
        bo_bc = pp.tile([128, QD], f32)                # bias broadcast
        ones_sb = pp.tile([1, 128], bf16)

        with (
            tc.tile_pool(name="inputs", bufs=1) as pin,
            tc.tile_pool(name="psA", bufs=2, space=bass.MemorySpace.PSUM) as psA,
        ):
            # ---- input tiles (freed after projections)
            xT_sb = pin.tile([128, KC_Q, NQL], bf16)       # 4 MB
            wq_sb = pin.tile([128, KC_Q, QD], bf16)        # 2 MB
            ctxT_sb = pin.tile([128, KC_C, NK], bf16)      # 1.5 MB
            wk_sb = pin.tile([128, KC_C, QD], bf16)
            wv_sb = pin.tile([128, KC_C, QD], bf16)
            bo_sb = pin.tile([1, QD], bf16)

            nc.sync.dma_start(ctxT_sb[:], ctxT_d)
            nc.sync.dma_start(wk_sb[:], wk_d)
            nc.sync.dma_start(wv_sb[:], wv_d)
            nc.sync.dma_start(xT_sb[:], xT_d)
            nc.sync.dma_start(wq_sb[:], wq_d)
            nc.sync.dma_start(wo_sb[:], wo_d)
            nc.sync.dma_start(bo_sb[:], bo_h[:])

            nc.vector.memset(ones_sb[:], 1.0)
            nc.vector.memset(vp_sb[:, :, :, HD], 1.0)      # ones column per head
            # broadcast bo across partitions via PE (ones outer product, bf16)
            for no in range(QD // 512):
                psb = psA.tile([128, 512], f32, tag="psA")
                nc.tensor.matmul(psb[:], ones_sb[:],
                                 bo_sb[0:1, no * 512:(no + 1) * 512],
                                 start=True, stop=True)
                nc.scalar.copy(bo_bc[:, no * 512:(no + 1) * 512], psb[:])

            # ---- phase K: KT[qd, k]
            for mo in range(KC_Q):
                for nk in range(NK // 512):
                    ps = psA.tile([128, 512], f32, tag="psA")
                    for c in range(KC_C):
                        nc.tensor.matmul(
                            ps[:],
                            wk_sb[:, c, mo * 128:(mo + 1) * 128],
                            ctxT_sb[:, c, nk * 512:(nk + 1) * 512],
                            start=(c == 0), stop=(c == KC_C - 1),
                        )
                    nc.vector.tensor_copy(
                        kt_sb[:, mo, nk * 512:(nk + 1) * 512], ps[:])

            # ---- phase V: V[k, qd] (+ones col) strided into vp_sb
            for ko in range(NKC):
                for nv in range(QD // 512):
                    ps = psA.tile([128, 512], f32, tag="psA")
                    for c in range(KC_C):
                        nc.tensor.matmul(
                            ps[:],
                            ctxT_sb[:, c, ko * 128:(ko + 1) * 128],
                            wv_sb[:, c, nv * 512:(nv + 1) * 512],
                            start=(c == 0), stop=(c == KC_C - 1),
                        )
                    nc.vector.tensor_copy(
                        vp_sb[:, ko, nv * 8:(nv + 1) * 8, 0:HD],
                        ps[:].rearrange("p (h d) -> p h d", h=8),
                    )

            # ---- phase Q: QT[qd, q]
            for mo in range(KC_Q):
                for nq in range(NQT):
                    ps = psA.tile([128, 512], f32, tag="psA")
                    for c in range(KC_Q):
                        nc.tensor.matmul(
                            ps[:],
                            wq_sb[:, c, mo * 128:(mo + 1) * 128],
                            xT_sb[:, c, nq * 512:(nq + 1) * 512],
                            start=(c == 0), stop=(c == KC_Q - 1),
                        )
                    nc.vector.tensor_copy(
                        qt_sb[:, mo, nq * 512:(nq + 1) * 512], ps[:])

        # ---- main attention loop
        with (
            tc.tile_pool(name="et", bufs=2) as pe_pool,
            tc.tile_pool(name="rsmall", bufs=2) as prs,
            tc.tile_pool(name="psS", bufs=2, space=bass.MemorySpace.PSUM) as psS,
            tc.tile_pool(name="psO", bufs=1, space=bass.MemorySpace.PSUM) as psO,
            tc.tile_pool(name="psR", bufs=1, space=bass.MemorySpace.PSUM) as psR,
        ):
            for hp in range(HP):
                h0, h1 = 2 * hp, 2 * hp + 1
                for qt in range(NQT):
                    qsl = slice(qt * 512, (qt + 1) * 512)
                    et0 = pe_pool.tile([128, NKC, 512], bf16, tag="et0")
                    et1 = pe_pool.tile([128, NKC, 512], bf16, tag="et1")
                    # ST = scores^T for both heads, row-tiled in one array pass
                    for kc in range(NKC):
                        ks = slice(kc * 128, (kc + 1) * 128)
                        ps0 = psS.tile([128, 512], f32, tag="ps0")
                        ps1 = psS.tile([128, 512], f32, tag="ps1")
                        nc.tensor.matmul(
                            ps0[:], kt_sb[0:64, hp, ks], qt_sb[0:64, hp, qsl],
                            start=True, stop=True, tile_position=(0, 0),
                        )
                        nc.tensor.matmul(
                            ps1[:], kt_sb[64:128, hp, ks], qt_sb[64:128, hp, qsl],
                            start=True, stop=True, tile_position=(64, 0),
                        )
                        nc.scalar.activation(et0[:, kc, :], ps0[:], EXP, scale=SCALE)
                        nc.scalar.activation(et1[:, kc, :], ps1[:], EXP, scale=SCALE)
                    # O'T = V'^T @ ET ; row 64 collects softmax denominators
                    po0 = psO.tile([HD + 1, 512], f32, tag="po0")
                    po1 = psO.tile([HD + 1, 512], f32, tag="po1")
                    for kc in range(NKC):
                        nc.tensor.matmul(
                            po0[:], vp_sb[:, kc, h0, :], et0[:, kc, :],
                            start=(kc == 0), stop=(kc == NKC - 1),
                        )
                    for kc in range(NKC):
                        nc.tensor.matmul(
                            po1[:], vp_sb[:, kc, h1, :], et1[:, kc, :],
                            start=(kc == 0), stop=(kc == NKC - 1),
                        )
                    # normalize: recip of denoms, broadcast across partitions via PE
                    r0 = prs.tile([1, 512], bf16, tag="r0")
                    r1 = prs.tile([1, 512], bf16, tag="r1")
                    with nc.allow_low_precision("softmax recip fits bf16"):
                        nc.vector.reciprocal(r0[:], po0[HD:HD + 1, :])
                        nc.vector.reciprocal(r1[:], po1[HD:HD + 1, :])
                    rb = psR.tile([128, 512], f32, tag="rb")
                    nc.tensor.matmul(rb[0:64, :], ones_sb[0:1, 0:64], r0[:],
                                     start=True, stop=True, tile_position=(0, 0))
                    nc.tensor.matmul(rb[64:128, :], ones_sb[0:1, 0:64], r1[:],
                                     start=True, stop=True, tile_position=(0, 64))
                    rs = prs.tile([128, 512], f32, tag="rs")
                    nc.scalar.copy(rs[:], rb[:])
                    nc.vector.tensor_mul(
                        attnT_sb[0:64, hp, qsl], po0[0:HD, :], rs[0:64, :])
                    nc.vector.tensor_mul(
                        attnT_sb[64:128, hp, qsl], po1[0:HD, :], rs[64:128, :])

        # ---- output projection: out[q, od] = attnT^T @ Wo + bo
        with (
            tc.tile_pool(name="yout", bufs=3) as py,
            tc.tile_pool(name="psY", bufs=2, space=bass.MemorySpace.PSUM) as psY,
        ):
            for mo in range(NQL // 128):
                for no in range(QD // 512):
                    ps = psY.tile([128, 512], f32, tag="psY")
                    for c in range(KC_Q):
                        nc.tensor.matmul(
                            ps[:],
                            attnT_sb[:, c, mo * 128:(mo + 1) * 128],
                            wo_sb[:, c, no * 512:(no + 1) * 512],
                            start=(c == 0), stop=(c == KC_Q - 1),
                        )
                    y = py.tile([128, 512], f32, tag="y")
                    nc.vector.tensor_add(
                        y[:], ps[:], bo_bc[:, no * 512:(no + 1) * 512])
                    nc.sync.dma_start(
                        out_d[:, mo, no * 512:(no + 1) * 512], y[:])

        _pp_cm.__exit__(None, None, None)

    nc.finalize()
    return nc


def make_in_maps(x, context, Wq, Wk, Wv, Wo, bo):
    """Host-side sharding + layout prep: transpose and cast to bf16."""
    import ml_dtypes
    bf16 = ml_dtypes.bfloat16

    x = np.asarray(x, np.float32)
    context = np.asarray(context, np.float32)
    wq = np.asarray(Wq, np.float32).astype(bf16)
    wk = np.asarray(Wk, np.float32).astype(bf16)
    wv = np.asarray(Wv, np.float32).astype(bf16)
    wo = np.asarray(Wo, np.float32).astype(bf16)
    bo = np.asarray(bo, np.float32).reshape(1, QD).astype(bf16)

    in_maps = []
    for c in range(N_CORES):
        b, half = c // 2, c % 2
        xs = x[b, half * NQL:(half + 1) * NQL, :]           # [2048, 1024]
        in_maps.append({
            "xT": np.ascontiguousarray(xs.T).astype(bf16),   # [1024, 2048]
            "ctxT": np.ascontiguousarray(context[b].T).astype(bf16),  # [768, 1024]
            "wq": wq, "wk": wk, "wv": wv, "wo": wo, "bo": bo,
        })
    return in_maps


_NC_CACHE = {}


def kernel(x, context, Wq, Wk, Wv, Wo, bo, _trace=False):
    import sys
    if "/opt/trn_rl_repo" not in sys.path:
        sys.path.insert(0, "/opt/trn_rl_repo")
    from concourse.bass_utils import run_bass_kernel_spmd

    if "nc" not in _NC_CACHE:
        _NC_CACHE["nc"] = build_bass()
    nc = _NC_CACHE["nc"]

    in_maps = make_in_maps(x, context, Wq, Wk, Wv, Wo, bo)
    res = run_bass_kernel_spmd(
        nc, in_maps, core_ids=list(range(N_CORES)), trace=_trace)

    out = np.empty((B, NQ, QD), np.float32)
    for c in range(N_CORES):
        b, half = c // 2, c % 2
        out[b, half * NQL:(half + 1) * NQL, :] = res.results[c]["out"]
    if _trace:
        return out, res
    return out


# revision 15
# speedup vs baseline: 1.0645x; 1.0645x over previous
"""Cross-attention Trainium2 kernel, 8-core data-parallel.

Problem (hardcoded): B=4, NQ=4096, NK=1024, QD=1024, CD=768, H=16, HD=64.
  out = softmax((x@Wq) @ (ctx@Wk)^T / sqrt(HD)) @ (ctx@Wv) @ Wo + bo

Sharding: pure data-parallel. 8 cores = 4 batches x 2 NQ-halves of 2048
query rows. Each core redundantly computes K/V projections for its batch
(3.2+3.2 GF vs 18.8 GF total per core -- cheap) and needs no collectives.

Per-core layout strategy: every matmul keeps its contraction dim on SBUF
partitions, so the chain is computed fully "transposed":
  QT[qd,q]   = Wq^T-contract(xT)        (x pre-transposed on host)
  KT[qd,k]   = Wk^T-contract(ctxT)
  V'[k,qd+1] = ctx@Wv with a ones column appended per head
  ST[k,q]    = per head: KT_h^T-slices @ QT_h (row-tiled 2 heads/matmul)
  ET         = exp(ST * 1/8)            (ACT, scale folded in)
  O'T[65,q]  = V'_h^T @ ET_h            (row 64 = softmax denominators)
  attnT      = O'T[0:64] * recip(denoms) (DVE, recip broadcast via PE)
  out[q,od]  = attnT^T @ Wo + bo
"""

import numpy as np

B, NQ, NK = 4, 4096, 1024
QD, CD, H = 1024, 768, 16
HD = QD // H
SCALE = HD ** -0.5
NQL = NQ // 2          # query rows per core
N_CORES = 8


def build_bass():
    """Build the per-core Bass graph (SPMD, identical on all 8 cores)."""
    import concourse.bass as bass
    import concourse.tile as tile
    from concourse import bacc, mybir

    f32 = mybir.dt.float32
    bf16 = mybir.dt.bfloat16
    EXP = mybir.ActivationFunctionType.Exp

    nc = bacc.Bacc()

    xT_h = nc.dram_tensor("xT", (QD, NQL), bf16, kind="ExternalInput")
    ctxT_h = nc.dram_tensor("ctxT", (CD, NK), bf16, kind="ExternalInput")
    wq_h = nc.dram_tensor("wq", (QD, QD), bf16, kind="ExternalInput")
    wk_h = nc.dram_tensor("wk", (CD, QD), bf16, kind="ExternalInput")
    wv_h = nc.dram_tensor("wv", (CD, QD), bf16, kind="ExternalInput")
    wo_h = nc.dram_tensor("wo", (QD, QD), bf16, kind="ExternalInput")
    bo_h = nc.dram_tensor("bo", (1, QD), bf16, kind="ExternalInput")
    out_h = nc.dram_tensor("out", (NQL, QD), f32, kind="ExternalOutput")

    # DRAM views chunked to 128 partitions
    xT_d = xT_h[:].rearrange("(c p) n -> p c n", p=128)       # [128, 8, 2048]
    ctxT_d = ctxT_h[:].rearrange("(c p) n -> p c n", p=128)   # [128, 6, 1024]
    wq_d = wq_h[:].rearrange("(c p) m -> p c m", p=128)       # [128, 8, 1024]
    wk_d = wk_h[:].rearrange("(c p) m -> p c m", p=128)       # [128, 6, 1024]
    wv_d = wv_h[:].rearrange("(c p) m -> p c m", p=128)       # [128, 6, 1024]
    wo_d = wo_h[:].rearrange("(c p) m -> p c m", p=128)       # [128, 8, 1024]
    out_d = out_h[:].rearrange("(t p) n -> p t n", p=128)     # [128, 16, 1024]

    KC_Q = QD // 128   # 8  contraction chunks for Q/out projections
    KC_C = CD // 128   # 6  contraction chunks for K/V projections
    NKC = NK // 128    # 8  key chunks
    NQT = NQL // 512   # 4  query tiles of 512
    HP = H // 2        # 8  head pairs

    with tile.TileContext(nc) as tc:
        _pp_cm = tc.tile_pool(name="persist", bufs=1)
        pp = _pp_cm.__enter__()
        # ---- persistent tiles (live for the whole kernel)
        qt_sb = pp.tile([128, KC_Q, NQL], bf16)        # QT   4 MB
        kt_sb = pp.tile([128, KC_Q, NK], bf16)         # KT   2 MB
        vp_sb = pp.tile([128, NKC, H, HD + 1], bf16)   # V'   2.08 MB
        attnT_sb = pp.tile([128, KC_Q, NQL], bf16)     # attn 4 MB
        wo_sb = pp.tile([128, KC_Q, QD], bf16)         # Wo   2 MB
        bo_bc = pp.tile([128, QD], f32)                # bias broadcast
        ones_sb = pp.tile([1, 128], bf16)

        with (
            tc.tile_pool(name="inputs", bufs=1) as pin,
            tc.tile_pool(name="psA", bufs=2, space=bass.MemorySpace.PSUM) as psA,
        ):
            # ---- input tiles (freed after projections)
            xT_sb = pin.tile([128, KC_Q, NQL], bf16)       # 4 MB
            wq_sb = pin.tile([128, KC_Q, QD], bf16)        # 2 MB
            ctxT_sb = pin.tile([128, KC_C, NK], bf16)      # 1.5 MB
            wk_sb = pin.tile([128, KC_C, QD], bf16)
            wv_sb = pin.tile([128, KC_C, QD], bf16)
            bo_sb = pin.tile([1, QD], bf16)

            nc.sync.dma_start(ctxT_sb[:], ctxT_d)
            nc.sync.dma_start(wk_sb[:], wk_d)
            nc.sync.dma_start(wv_sb[:], wv_d)
            nc.sync.dma_start(xT_sb[:], xT_d)
            nc.sync.dma_start(wq_sb[:], wq_d)
            nc.sync.dma_start(wo_sb[:], wo_d)
            nc.sync.dma_start(bo_sb[:], bo_h[:])

            nc.vector.memset(ones_sb[:], 1.0)
            nc.vector.memset(vp_sb[:, :, :, HD], 1.0)      # ones column per head
            # broadcast bo across partitions via PE (ones outer product, bf16)
            for no in range(QD // 512):
                psb = psA.tile([128, 512], f32, tag="psA")
                nc.tensor.matmul(psb[:], ones_sb[:],
                                 bo_sb[0:1, no * 512:(no + 1) * 512],
                                 start=True, stop=True)
                nc.scalar.copy(bo_bc[:, no * 512:(no + 1) * 512], psb[:])

            # ---- phase K: KT[qd, k]
            for mo in range(KC_Q):
                for nk in range(NK // 512):
                    ps = psA.tile([128, 512], f32, tag="psA")
                    for c in range(KC_C):
                        nc.tensor.matmul(
                            ps[:],
                            wk_sb[:, c, mo * 128:(mo + 1) * 128],
                            ctxT_sb[:, c, nk * 512:(nk + 1) * 512],
                            start=(c == 0), stop=(c == KC_C - 1),
                        )
                    nc.vector.tensor_copy(
                        kt_sb[:, mo, nk * 512:(nk + 1) * 512], ps[:])

            # ---- phase V: V[k, qd] (+ones col) strided into vp_sb
            for ko in range(NKC):
                for nv in range(QD // 512):
                    ps = psA.tile([128, 512], f32, tag="psA")
                    for c in range(KC_C):
                        nc.tensor.matmul(
                            ps[:],
                            ctxT_sb[:, c, ko * 128:(ko + 1) * 128],
                            wv_sb[:, c, nv * 512:(nv + 1) * 512],
                            start=(c == 0), stop=(c == KC_C - 1),
                        )
                    nc.vector.tensor_copy(
                        vp_sb[:, ko, nv * 8:(nv + 1) * 8, 0:HD],
                        ps[:].rearrange("p (h d) -> p h d", h=8),
                    )

            # ---- phase Q: QT[qd, q]
            for mo in range(KC_Q):
                for nq in range(NQT):
                    ps = psA.tile([128, 512], f32, tag="psA")
                    for c in range(KC_Q):
                        nc.tensor.matmul(
                            ps[:],
                            wq_sb[:, c, mo * 128:(mo + 1) * 128],
                            xT_sb[:, c, nq * 512:(nq + 1) * 512],
                            start=(c == 0), stop=(c == KC_Q - 1),
                        )
                    nc.vector.tensor_copy(
                        qt_sb[:, mo, nq * 512:(nq + 1) * 512], ps[:])

        # ---- main loop: qt2-outer (1024-wide q tiles), head-pairs inner.
        # Wide F=2048 exp tiles (both heads share one 4-bank PSUM stage);
        # the output projection for each qt2's columns runs as PE filler
        # interleaved with the next qt2's attention.
        with (
            tc.tile_pool(name="et", bufs=2) as pe_pool,
            tc.tile_pool(name="rsmall", bufs=4) as prs,
            tc.tile_pool(name="yout", bufs=3) as py,
            tc.tile_pool(name="psS", bufs=1, space=bass.MemorySpace.PSUM) as psS,
            tc.tile_pool(name="psO", bufs=2, space=bass.MemorySpace.PSUM) as psO,
            tc.tile_pool(name="psR", bufs=1, space=bass.MemorySpace.PSUM) as psR,
            tc.tile_pool(name="psX", bufs=1, space=bass.MemorySpace.PSUM) as psX,
        ):
            def y_cols(mo):
                # out rows [128mo : 128mo+128] = attnT^T @ Wo + bo
                for no in range(QD // 512):
                    ps = psX.tile([128, 512], f32, tag="psX")
                    for c in range(KC_Q):
                        nc.tensor.matmul(
                            ps[:],
                            attnT_sb[:, c, mo * 128:(mo + 1) * 128],
                            wo_sb[:, c, no * 512:(no + 1) * 512],
                            start=(c == 0), stop=(c == KC_Q - 1),
                        )
                    y = py.tile([128, 512], f32, tag="y")
                    nc.vector.tensor_add(
                        y[:], ps[:], bo_bc[:, no * 512:(no + 1) * 512])
                    nc.sync.dma_start(
                        out_d[:, mo, no * 512:(no + 1) * 512], y[:])

            for qt2 in range(2):
                for hp in range(HP):
                    h0, h1 = 2 * hp, 2 * hp + 1
                    q0 = qt2 * 1024
                    # ET pair layout: [128, kc, h0 cols 0:1024 | h1 1024:2048]
                    etp = pe_pool.tile([128, NKC, 2048], bf16, tag="etp")
                    for kc in range(NKC):
                        ks = slice(kc * 128, (kc + 1) * 128)
                        ps = psS.tile([128, 2048], f32, tag="psS")
                        for qh in range(2):
                            qsl = slice(q0 + qh * 512, q0 + (qh + 1) * 512)
                            nc.tensor.matmul(
                                ps[:, qh * 512:(qh + 1) * 512],
                                kt_sb[0:64, hp, ks], qt_sb[0:64, hp, qsl],
                                start=True, stop=True, tile_position=(0, 0),
                            )
                            nc.tensor.matmul(
                                ps[:, 1024 + qh * 512:1024 + (qh + 1) * 512],
                                kt_sb[64:128, hp, ks], qt_sb[64:128, hp, qsl],
                                start=True, stop=True, tile_position=(64, 0),
                            )
                        # one wide exp per k-chunk (both heads, 1024 q cols)
                        nc.scalar.activation(etp[:, kc, :], ps[:], EXP, scale=SCALE)
                    # O'T = V'^T @ ET per head & q-half; row 64 = denominators
                    for h_i, h in enumerate((h0, h1)):
                        for qh in range(2):
                            po = psO.tile([HD + 1, 512], f32, tag="po")
                            esl = slice(h_i * 1024 + qh * 512,
                                        h_i * 1024 + (qh + 1) * 512)
                            for kc in range(NKC):
                                nc.tensor.matmul(
                                    po[:], vp_sb[:, kc, h, :], etp[:, kc, esl],
                                    start=(kc == 0), stop=(kc == NKC - 1),
                                )
                            r = prs.tile([1, 512], bf16, tag="r")
                            with nc.allow_low_precision("softmax recip in bf16"):
                                nc.vector.reciprocal(r[:], po[HD:HD + 1, :])
                            rb = psR.tile([64, 512], f32, tag="rb")
                            nc.tensor.matmul(rb[:], ones_sb[0:1, 0:64], r[:],
                                             start=True, stop=True)
                            rs = prs.tile([64, 512], f32, tag="rs")
                            nc.vector.tensor_copy(rs[:], rb[:])
                            prow = slice(h_i * 64, h_i * 64 + 64)
                            qsl = slice(q0 + qh * 512, q0 + (qh + 1) * 512)
                            nc.vector.tensor_mul(
                                attnT_sb[prow, hp, qsl], po[0:HD, :], rs[:])
                # output projection for this qt2's q columns (PE filler)
                for mo in range(qt2 * 8, qt2 * 8 + 8):
                    y_cols(mo)

        _pp_cm.__exit__(None, None, None)

    nc.finalize()
    return nc


def make_in_maps(x, context, Wq, Wk, Wv, Wo, bo):
    """Host-side sharding + layout prep: transpose and cast to bf16."""
    import ml_dtypes
    bf16 = ml_dtypes.bfloat16

    x = np.asarray(x, np.float32)
    context = np.asarray(context, np.float32)
    wq = np.asarray(Wq, np.float32).astype(bf16)
    wk = np.asarray(Wk, np.float32).astype(bf16)
    wv = np.asarray(Wv, np.float32).astype(bf16)
    wo = np.asarray(Wo, np.float32).astype(bf16)
    bo = np.asarray(bo, np.float32).reshape(1, QD).astype(bf16)

    in_maps = []
    for c in range(N_CORES):
        b, half = c // 2, c % 2
        xs = x[b, half * NQL:(half + 1) * NQL, :]           # [2048, 1024]
        in_maps.append({
            "xT": np.ascontiguousarray(xs.T).astype(bf16),   # [1024, 2048]
            "ctxT": np.ascontiguousarray(context[b].T).astype(bf16),  # [768, 1024]
            "wq": wq, "wk": wk, "wv": wv, "wo": wo, "bo": bo,
        })
    return in_maps


_NC_CACHE = {}


def kernel(x, context, Wq, Wk, Wv, Wo, bo, _trace=False):
    import sys
    if "/opt/trn_rl_repo" not in sys.path:
        sys.path.insert(0, "/opt/trn_rl_repo")
    from concourse.bass_utils import run_bass_kernel_spmd

    if "nc" not in _NC_CACHE:
        _NC_CACHE["nc"] = build_bass()
    nc = _NC_CACHE["nc"]

    in_maps = make_in_maps(x, context, Wq, Wk, Wv, Wo, bo)
    res = run_bass_kernel_spmd(
        nc, in_maps, core_ids=list(range(N_CORES)), trace=_trace)

    out = np.empty((B, NQ, QD), np.float32)
    for c in range(N_CORES):
        b, half = c // 2, c % 2
        out[b, half * NQL:(half + 1) * NQL, :] = res.results[c]["out"]
    if _trace:
        return out, res
    return out


# revision 20
# speedup vs baseline: 1.2325x; 1.1578x over previous
"""Cross-attention Trainium2 kernel, 8-core data-parallel.

Problem (hardcoded): B=4, NQ=4096, NK=1024, QD=1024, CD=768, H=16, HD=64.
  out = softmax((x@Wq) @ (ctx@Wk)^T / sqrt(HD)) @ (ctx@Wv) @ Wo + bo

Sharding: pure data-parallel. 8 cores = 4 batches x 2 NQ-halves of 2048
query rows. Each core redundantly computes K/V projections for its batch
(3.2+3.2 GF vs 18.8 GF total per core -- cheap) and needs no collectives.

Per-core layout strategy: every matmul keeps its contraction dim on SBUF
partitions, so the chain is computed fully "transposed":
  QT[qd,q]   = Wq^T-contract(xT)        (x pre-transposed on host)
  KT[qd,k]   = Wk^T-contract(ctxT)
  V'[k,qd+1] = ctx@Wv with a ones column appended per head
  ST[k,q]    = per head: KT_h^T-slices @ QT_h (row-tiled 2 heads/matmul)
  ET         = exp(ST * 1/8)            (ACT, scale folded in)
  O'T[65,q]  = V'_h^T @ ET_h            (row 64 = softmax denominators)
  attnT      = O'T[0:64] * recip(denoms) (DVE, recip broadcast via PE)
  out[q,od]  = attnT^T @ Wo + bo
"""

import numpy as np

B, NQ, NK = 4, 4096, 1024
QD, CD, H = 1024, 768, 16
HD = QD // H
SCALE = HD ** -0.5
NQL = NQ // 2          # query rows per core
N_CORES = 8


def build_bass():
    """Build the per-core Bass graph (SPMD, identical on all 8 cores)."""
    import concourse.bass as bass
    import concourse.tile as tile
    from concourse import bacc, mybir

    f32 = mybir.dt.float32
    bf16 = mybir.dt.bfloat16
    EXP = mybir.ActivationFunctionType.Exp

    nc = bacc.Bacc()

    xT_h = nc.dram_tensor("xT", (QD, NQL), bf16, kind="ExternalInput")
    ctxT_h = nc.dram_tensor("ctxT", (CD, NK), bf16, kind="ExternalInput")
    wq_h = nc.dram_tensor("wq", (QD, QD), bf16, kind="ExternalInput")
    wk_h = nc.dram_tensor("wk", (CD, QD), bf16, kind="ExternalInput")
    wv_h = nc.dram_tensor("wv", (CD, QD), bf16, kind="ExternalInput")
    wo_h = nc.dram_tensor("wo", (QD, QD), bf16, kind="ExternalInput")
    bo_h = nc.dram_tensor("bo", (1, QD), bf16, kind="ExternalInput")
    out_h = nc.dram_tensor("out", (NQL, QD), f32, kind="ExternalOutput")

    # DRAM views chunked to 128 partitions
    xT_d = xT_h[:].rearrange("(c p) n -> p c n", p=128)       # [128, 8, 2048]
    ctxT_d = ctxT_h[:].rearrange("(c p) n -> p c n", p=128)   # [128, 6, 1024]
    wq_d = wq_h[:].rearrange("(c p) m -> p c m", p=128)       # [128, 8, 1024]
    wk_d = wk_h[:].rearrange("(c p) m -> p c m", p=128)       # [128, 6, 1024]
    wv_d = wv_h[:].rearrange("(c p) m -> p c m", p=128)       # [128, 6, 1024]
    wo_d = wo_h[:].rearrange("(c p) m -> p c m", p=128)       # [128, 8, 1024]
    out_d = out_h[:].rearrange("(t p) n -> p t n", p=128)     # [128, 16, 1024]

    KC_Q = QD // 128   # 8  contraction chunks for Q/out projections
    KC_C = CD // 128   # 6  contraction chunks for K/V projections
    NKC = NK // 128    # 8  key chunks
    NQT = NQL // 512   # 4  query tiles of 512
    HP = H // 2        # 8  head pairs

    with tile.TileContext(nc) as tc:
        _pp_cm = tc.tile_pool(name="persist", bufs=1)
        pp = _pp_cm.__enter__()
        # ---- persistent tiles (live for the whole kernel)
        qt_sb = pp.tile([128, KC_Q, NQL], bf16)        # QT   4 MB
        kt_sb = pp.tile([128, KC_Q, NK], bf16)         # KT   2 MB
        vp_sb = pp.tile([128, NKC, H, HD + 1], bf16)   # V'   2.08 MB
        attnT_sb = pp.tile([128, KC_Q, NQL], bf16)     # attn 4 MB
        wo_sb = pp.tile([128, KC_Q, QD], bf16)         # Wo   2 MB
        bo_bc = pp.tile([128, QD], f32)                # bias broadcast
        ones_sb = pp.tile([1, 128], bf16)

        with (
            tc.tile_pool(name="inputs", bufs=1) as pin,
            tc.tile_pool(name="psA", bufs=2, space=bass.MemorySpace.PSUM) as psA,
        ):
            # ---- input tiles (freed after projections)
            xT_sb = pin.tile([128, KC_Q, NQL], bf16)       # 4 MB
            wq_sb = pin.tile([128, KC_Q, QD], bf16)        # 2 MB
            ctxT_sb = pin.tile([128, KC_C, NK], bf16)      # 1.5 MB
            wk_sb = pin.tile([128, KC_C, QD], bf16)
            wv_sb = pin.tile([128, KC_C, QD], bf16)
            bo_sb = pin.tile([1, QD], bf16)

            nc.sync.dma_start(ctxT_sb[:], ctxT_d)
            nc.sync.dma_start(wk_sb[:], wk_d)
            nc.sync.dma_start(wv_sb[:], wv_d)
            nc.sync.dma_start(xT_sb[:], xT_d)
            nc.sync.dma_start(wq_sb[:], wq_d)
            nc.sync.dma_start(wo_sb[:], wo_d)
            nc.sync.dma_start(bo_sb[:], bo_h[:])

            nc.vector.memset(ones_sb[:], 1.0)
            nc.vector.memset(vp_sb[:, :, :, HD], 1.0)      # ones column per head
            # broadcast bo across partitions via PE (ones outer product, bf16)
            for no in range(QD // 512):
                psb = psA.tile([128, 512], f32, tag="psA")
                nc.tensor.matmul(psb[:], ones_sb[:],
                                 bo_sb[0:1, no * 512:(no + 1) * 512],
                                 start=True, stop=True)
                nc.scalar.copy(bo_bc[:, no * 512:(no + 1) * 512], psb[:])

            # ---- phase K: KT[qd, k]
            for mo in range(KC_Q):
                for nk in range(NK // 512):
                    ps = psA.tile([128, 512], f32, tag="psA")
                    for c in range(KC_C):
                        nc.tensor.matmul(
                            ps[:],
                            wk_sb[:, c, mo * 128:(mo + 1) * 128],
                            ctxT_sb[:, c, nk * 512:(nk + 1) * 512],
                            start=(c == 0), stop=(c == KC_C - 1),
                        )
                    if (mo + nk) % 2 == 0:
                        nc.vector.tensor_copy(
                            kt_sb[:, mo, nk * 512:(nk + 1) * 512], ps[:])
                    else:
                        nc.scalar.copy(
                            kt_sb[:, mo, nk * 512:(nk + 1) * 512], ps[:])

            # ---- phase V: V[k, qd] (+ones col) strided into vp_sb
            for ko in range(NKC):
                for nv in range(QD // 512):
                    ps = psA.tile([128, 512], f32, tag="psA")
                    for c in range(KC_C):
                        nc.tensor.matmul(
                            ps[:],
                            ctxT_sb[:, c, ko * 128:(ko + 1) * 128],
                            wv_sb[:, c, nv * 512:(nv + 1) * 512],
                            start=(c == 0), stop=(c == KC_C - 1),
                        )
                    if (ko + nv) % 2 == 0:
                        nc.vector.tensor_copy(
                            vp_sb[:, ko, nv * 8:(nv + 1) * 8, 0:HD],
                            ps[:].rearrange("p (h d) -> p h d", h=8),
                        )
                    else:
                        nc.scalar.copy(
                            vp_sb[:, ko, nv * 8:(nv + 1) * 8, 0:HD],
                            ps[:].rearrange("p (h d) -> p h d", h=8),
                        )

            # ---- phase Q: QT[qd, q]
            for mo in range(KC_Q):
                for nq in range(NQT):
                    ps = psA.tile([128, 512], f32, tag="psA")
                    for c in range(KC_Q):
                        nc.tensor.matmul(
                            ps[:],
                            wq_sb[:, c, mo * 128:(mo + 1) * 128],
                            xT_sb[:, c, nq * 512:(nq + 1) * 512],
                            start=(c == 0), stop=(c == KC_Q - 1),
                        )
                    if (mo + nq) % 2 == 0:
                        nc.vector.tensor_copy(
                            qt_sb[:, mo, nq * 512:(nq + 1) * 512], ps[:])
                    else:
                        nc.scalar.copy(
                            qt_sb[:, mo, nq * 512:(nq + 1) * 512], ps[:])

        # ---- main loop: qt2-outer (1024-wide q tiles), head-pairs inner.
        # Wide F=2048 exp tiles (both heads share one 4-bank PSUM stage);
        # the output projection for each qt2's columns runs as PE filler
        # interleaved with the next qt2's attention.
        with (
            tc.tile_pool(name="et", bufs=2) as pe_pool,
            tc.tile_pool(name="rsmall", bufs=4) as prs,
            tc.tile_pool(name="yout", bufs=3) as py,
            tc.tile_pool(name="psS", bufs=2, space=bass.MemorySpace.PSUM) as psS,
            tc.tile_pool(name="psO", bufs=2, space=bass.MemorySpace.PSUM) as psO,
            tc.tile_pool(name="psR", bufs=1, space=bass.MemorySpace.PSUM) as psR,
            tc.tile_pool(name="psX", bufs=1, space=bass.MemorySpace.PSUM) as psX,
        ):
            def y_cols(mo):
                # out rows [128mo : 128mo+128] = attnT^T @ Wo + bo
                for no in range(QD // 512):
                    ps = psX.tile([128, 512], f32, tag="psX")
                    for c in range(KC_Q):
                        nc.tensor.matmul(
                            ps[:],
                            attnT_sb[:, c, mo * 128:(mo + 1) * 128],
                            wo_sb[:, c, no * 512:(no + 1) * 512],
                            start=(c == 0), stop=(c == KC_Q - 1),
                        )
                    y = py.tile([128, 512], f32, tag="y")
                    nc.vector.tensor_add(
                        y[:], ps[:], bo_bc[:, no * 512:(no + 1) * 512])
                    nc.sync.dma_start(
                        out_d[:, mo, no * 512:(no + 1) * 512], y[:])

            for qt2 in range(2):
                for hp in range(HP):
                    h0, h1 = 2 * hp, 2 * hp + 1
                    q0 = qt2 * 1024
                    # ET layout: [128, kc, qh, h0 cols 0:512 | h1 cols 512:1024]
                    etp = pe_pool.tile([128, NKC, 2, 1024], bf16, tag="etp")
                    for kc in range(NKC):
                        ks = slice(kc * 128, (kc + 1) * 128)
                        for qh in range(2):
                            qsl = slice(q0 + qh * 512, q0 + (qh + 1) * 512)
                            ps = psS.tile([128, 1024], f32, tag="psS")
                            nc.tensor.matmul(
                                ps[:, 0:512],
                                kt_sb[0:64, hp, ks], qt_sb[0:64, hp, qsl],
                                start=True, stop=True, tile_position=(0, 0),
                            )
                            nc.tensor.matmul(
                                ps[:, 512:1024],
                                kt_sb[64:128, hp, ks], qt_sb[64:128, hp, qsl],
                                start=True, stop=True, tile_position=(64, 0),
                            )
                            # one exp per (k-chunk, q-half): both heads
                            nc.scalar.activation(
                                etp[:, kc, qh, :], ps[:], EXP, scale=SCALE)
                    # O'T = V'^T @ ET per head & q-half; row 64 = denominators
                    for h_i, h in enumerate((h0, h1)):
                        for qh in range(2):
                            po = psO.tile([HD + 1, 512], f32, tag="po")
                            esl = slice(h_i * 512, (h_i + 1) * 512)
                            for kc in range(NKC):
                                nc.tensor.matmul(
                                    po[:], vp_sb[:, kc, h, :],
                                    etp[:, kc, qh, esl],
                                    start=(kc == 0), stop=(kc == NKC - 1),
                                )
                            r = prs.tile([1, 512], bf16, tag="r")
                            with nc.allow_low_precision("softmax recip in bf16"):
                                nc.vector.reciprocal(r[:], po[HD:HD + 1, :])
                            rb = psR.tile([64, 512], f32, tag="rb")
                            nc.tensor.matmul(rb[:], ones_sb[0:1, 0:64], r[:],
                                             start=True, stop=True)
                            rs = prs.tile([64, 512], f32, tag="rs")
                            nc.vector.tensor_copy(rs[:], rb[:])
                            prow = slice(h_i * 64, h_i * 64 + 64)
                            qsl = slice(q0 + qh * 512, q0 + (qh + 1) * 512)
                            nc.vector.tensor_mul(
                                attnT_sb[prow, hp, qsl], po[0:HD, :], rs[:])
                # output projection for this qt2's q columns (PE filler)
                for mo in range(qt2 * 8, qt2 * 8 + 8):
                    y_cols(mo)

        _pp_cm.__exit__(None, None, None)

    nc.finalize()
    return nc


def make_in_maps(x, context, Wq, Wk, Wv, Wo, bo):
    """Host-side sharding + layout prep: transpose and cast to bf16."""
    import ml_dtypes
    bf16 = ml_dtypes.bfloat16

    x = np.asarray(x, np.float32)
    context = np.asarray(context, np.float32)
    wq = np.asarray(Wq, np.float32).astype(bf16)
    wk = np.asarray(Wk, np.float32).astype(bf16)
    wv = np.asarray(Wv, np.float32).astype(bf16)
    wo = np.asarray(Wo, np.float32).astype(bf16)
    bo = np.asarray(bo, np.float32).reshape(1, QD).astype(bf16)

    in_maps = []
    for c in range(N_CORES):
        b, half = c // 2, c % 2
        xs = x[b, half * NQL:(half + 1) * NQL, :]           # [2048, 1024]
        in_maps.append({
            "xT": np.ascontiguousarray(xs.T).astype(bf16),   # [1024, 2048]
            "ctxT": np.ascontiguousarray(context[b].T).astype(bf16),  # [768, 1024]
            "wq": wq, "wk": wk, "wv": wv, "wo": wo, "bo": bo,
        })
    return in_maps


_NC_CACHE = {}


def kernel(x, context, Wq, Wk, Wv, Wo, bo, _trace=False):
    import sys
    if "/opt/trn_rl_repo" not in sys.path:
        sys.path.insert(0, "/opt/trn_rl_repo")
    from concourse.bass_utils import run_bass_kernel_spmd

    if "nc" not in _NC_CACHE:
        _NC_CACHE["nc"] = build_bass()
    nc = _NC_CACHE["nc"]

    in_maps = make_in_maps(x, context, Wq, Wk, Wv, Wo, bo)
    res = run_bass_kernel_spmd(
        nc, in_maps, core_ids=list(range(N_CORES)), trace=_trace)

    out = np.empty((B, NQ, QD), np.float32)
    for c in range(N_CORES):
        b, half = c // 2, c % 2
        out[b, half * NQL:(half + 1) * NQL, :] = res.results[c]["out"]
    if _trace:
        return out, res
    return out


# revision 24
# speedup vs baseline: 1.6360x; 1.3274x over previous
"""Cross-attention Trainium2 kernel, 8-core data-parallel.

Problem (hardcoded): B=4, NQ=4096, NK=1024, QD=1024, CD=768, H=16, HD=64.
  out = softmax((x@Wq) @ (ctx@Wk)^T / sqrt(HD)) @ (ctx@Wv) @ Wo + bo

Sharding: pure data-parallel. 8 cores = 4 batches x 2 NQ-halves of 2048
query rows. Each core redundantly computes K/V projections for its batch
(3.2+3.2 GF vs 18.8 GF total per core -- cheap) and needs no collectives.

Per-core layout strategy: every matmul keeps its contraction dim on SBUF
partitions, so the chain is computed fully "transposed":
  QT[qd,q]   = Wq^T-contract(xT)        (x pre-transposed on host)
  KT[qd,k]   = Wk^T-contract(ctxT)
  V'[k,qd+1] = ctx@Wv with a ones column appended per head
  ST[k,q]    = per head: KT_h^T-slices @ QT_h (row-tiled 2 heads/matmul)
  ET         = exp(ST * 1/8)            (ACT, scale folded in)
  O'T[65,q]  = V'_h^T @ ET_h            (row 64 = softmax denominators)
  attnT      = O'T[0:64] * recip(denoms) (DVE, recip broadcast via PE)
  out[q,od]  = attnT^T @ Wo + bo
"""

import numpy as np

B, NQ, NK = 4, 4096, 1024
QD, CD, H = 1024, 768, 16
HD = QD // H
SCALE = HD ** -0.5
NQL = NQ // 2          # query rows per core
N_CORES = 8


def build_bass():
    """Build the per-core Bass graph (SPMD, identical on all 8 cores)."""
    import concourse.bass as bass
    import concourse.tile as tile
    from concourse import bacc, mybir

    f32 = mybir.dt.float32
    bf16 = mybir.dt.bfloat16
    EXP = mybir.ActivationFunctionType.Exp

    nc = bacc.Bacc()

    xT_h = nc.dram_tensor("xT", (QD, NQL), bf16, kind="ExternalInput")
    ctxT_h = nc.dram_tensor("ctxT", (CD, NK), bf16, kind="ExternalInput")
    wq_h = nc.dram_tensor("wq", (QD, QD), bf16, kind="ExternalInput")
    wk_h = nc.dram_tensor("wk", (CD, QD), bf16, kind="ExternalInput")
    wv_h = nc.dram_tensor("wv", (CD, QD), bf16, kind="ExternalInput")
    wo_h = nc.dram_tensor("wo", (QD, QD), bf16, kind="ExternalInput")
    bo_h = nc.dram_tensor("bo", (1, QD), bf16, kind="ExternalInput")
    out_h = nc.dram_tensor("out", (NQL, QD), f32, kind="ExternalOutput")

    # DRAM views chunked to 128 partitions
    xT_d = xT_h[:].rearrange("(c p) n -> p c n", p=128)       # [128, 8, 2048]
    ctxT_d = ctxT_h[:].rearrange("(c p) n -> p c n", p=128)   # [128, 6, 1024]
    wq_d = wq_h[:].rearrange("(c p) m -> p c m", p=128)       # [128, 8, 1024]
    wk_d = wk_h[:].rearrange("(c p) m -> p c m", p=128)       # [128, 6, 1024]
    wv_d = wv_h[:].rearrange("(c p) m -> p c m", p=128)       # [128, 6, 1024]
    wo_d = wo_h[:].rearrange("(c p) m -> p c m", p=128)       # [128, 8, 1024]
    out_d = out_h[:].rearrange("(t p) n -> p t n", p=128)     # [128, 16, 1024]

    KC_Q = QD // 128   # 8  contraction chunks for Q/out projections
    KC_C = CD // 128   # 6  contraction chunks for K/V projections
    NKC = NK // 128    # 8  key chunks
    NQT = NQL // 512   # 4  query tiles of 512
    HP = H // 2        # 8  head pairs

    with tile.TileContext(nc) as tc:
        _pp_cm = tc.tile_pool(name="persist", bufs=1)
        pp = _pp_cm.__enter__()
        # ---- persistent tiles (live for the whole kernel)
        qt_sb = pp.tile([128, KC_Q, NQL], bf16)        # QT   4 MB
        kt_sb = pp.tile([128, KC_Q, NK], bf16)         # KT   2 MB
        vp_sb = pp.tile([128, NKC, H, HD + 1], bf16)   # V'   2.08 MB
        attnT_sb = pp.tile([128, KC_Q, NQL], bf16)     # attn 4 MB
        wo_sb = pp.tile([128, KC_Q, QD], bf16)         # Wo   2 MB
        bo_bc = pp.tile([128, QD], f32)                # bias broadcast
        ones_sb = pp.tile([1, 128], bf16)

        with (
            tc.tile_pool(name="inputs", bufs=1) as pin,
            tc.tile_pool(name="psA", bufs=2, space=bass.MemorySpace.PSUM) as psA,
        ):
            # ---- input tiles (freed after projections)
            xT_sb = pin.tile([128, KC_Q, NQL], bf16)       # 4 MB
            wq_sb = pin.tile([128, KC_Q, QD], bf16)        # 2 MB
            ctxT_sb = pin.tile([128, KC_C, NK], bf16)      # 1.5 MB
            wk_sb = pin.tile([128, KC_C, QD], bf16)
            wv_sb = pin.tile([128, KC_C, QD], bf16)
            bo_sb = pin.tile([1, QD], bf16)

            nc.sync.dma_start(ctxT_sb[:], ctxT_d)
            nc.sync.dma_start(wk_sb[:], wk_d)
            nc.sync.dma_start(wv_sb[:], wv_d)
            nc.sync.dma_start(xT_sb[:], xT_d)
            nc.sync.dma_start(wq_sb[:], wq_d)
            nc.sync.dma_start(wo_sb[:], wo_d)
            nc.sync.dma_start(bo_sb[:], bo_h[:])

            nc.vector.memset(ones_sb[:], 1.0)
            nc.vector.memset(vp_sb[:, :, :, HD], 1.0)      # ones column per head
            # broadcast bo across partitions via PE (ones outer product, bf16)
            for no in range(QD // 512):
                psb = psA.tile([128, 512], f32, tag="psA")
                nc.tensor.matmul(psb[:], ones_sb[:],
                                 bo_sb[0:1, no * 512:(no + 1) * 512],
                                 start=True, stop=True)
                nc.scalar.copy(bo_bc[:, no * 512:(no + 1) * 512], psb[:])

            # ---- phase K: KT[qd, k]
            for mo in range(KC_Q):
                for nk in range(NK // 512):
                    ps = psA.tile([128, 512], f32, tag="psA")
                    for c in range(KC_C):
                        nc.tensor.matmul(
                            ps[:],
                            wk_sb[:, c, mo * 128:(mo + 1) * 128],
                            ctxT_sb[:, c, nk * 512:(nk + 1) * 512],
                            start=(c == 0), stop=(c == KC_C - 1),
                        )
                    if (mo + nk) % 2 == 0:
                        nc.vector.tensor_copy(
                            kt_sb[:, mo, nk * 512:(nk + 1) * 512], ps[:])
                    else:
                        nc.scalar.copy(
                            kt_sb[:, mo, nk * 512:(nk + 1) * 512], ps[:])

            # ---- phase V: V[k, qd] (+ones col) strided into vp_sb
            for ko in range(NKC):
                for nv in range(QD // 512):
                    ps = psA.tile([128, 512], f32, tag="psA")
                    for c in range(KC_C):
                        nc.tensor.matmul(
                            ps[:],
                            ctxT_sb[:, c, ko * 128:(ko + 1) * 128],
                            wv_sb[:, c, nv * 512:(nv + 1) * 512],
                            start=(c == 0), stop=(c == KC_C - 1),
                        )
                    if (ko + nv) % 2 == 0:
                        nc.vector.tensor_copy(
                            vp_sb[:, ko, nv * 8:(nv + 1) * 8, 0:HD],
                            ps[:].rearrange("p (h d) -> p h d", h=8),
                        )
                    else:
                        nc.scalar.copy(
                            vp_sb[:, ko, nv * 8:(nv + 1) * 8, 0:HD],
                            ps[:].rearrange("p (h d) -> p h d", h=8),
                        )

            # ---- phase Q: QT[qd, q]
            for mo in range(KC_Q):
                for nq in range(NQT):
                    ps = psA.tile([128, 512], f32, tag="psA")
                    for c in range(KC_Q):
                        nc.tensor.matmul(
                            ps[:],
                            wq_sb[:, c, mo * 128:(mo + 1) * 128],
                            xT_sb[:, c, nq * 512:(nq + 1) * 512],
                            start=(c == 0), stop=(c == KC_Q - 1),
                        )
                    if (mo + nq) % 2 == 0:
                        nc.vector.tensor_copy(
                            qt_sb[:, mo, nq * 512:(nq + 1) * 512], ps[:])
                    else:
                        nc.scalar.copy(
                            qt_sb[:, mo, nq * 512:(nq + 1) * 512], ps[:])

        # ---- main loop: qt2-outer (1024-wide q tiles), head-pairs inner.
        # Wide F=2048 exp tiles (both heads share one 4-bank PSUM stage);
        # the output projection for each qt2's columns runs as PE filler
        # interleaved with the next qt2's attention.
        with (
            tc.tile_pool(name="et", bufs=2) as pe_pool,
            tc.tile_pool(name="rsmall", bufs=2) as prs,
            tc.tile_pool(name="yout", bufs=3) as py,
            tc.tile_pool(name="psS", bufs=2, space=bass.MemorySpace.PSUM) as psS,
            tc.tile_pool(name="psO", bufs=3, space=bass.MemorySpace.PSUM) as psO,
            tc.tile_pool(name="psX", bufs=1, space=bass.MemorySpace.PSUM) as psX,
        ):
            def y_cols(mo):
                # out rows [128mo : 128mo+128] = attnT^T @ Wo + bo
                for no in range(QD // 512):
                    ps = psX.tile([128, 512], f32, tag="psX")
                    for c in range(KC_Q):
                        nc.tensor.matmul(
                            ps[:],
                            attnT_sb[:, c, mo * 128:(mo + 1) * 128],
                            wo_sb[:, c, no * 512:(no + 1) * 512],
                            start=(c == 0), stop=(c == KC_Q - 1),
                        )
                    y = py.tile([128, 512], f32, tag="y")
                    nc.vector.tensor_add(
                        y[:], ps[:], bo_bc[:, no * 512:(no + 1) * 512])
                    nc.sync.dma_start(
                        out_d[:, mo, no * 512:(no + 1) * 512], y[:])

            for qt2 in range(2):
                for hp in range(HP):
                    h0, h1 = 2 * hp, 2 * hp + 1
                    q0 = qt2 * 1024
                    # ET layout: [128, kc, qh, h0 cols 0:512 | h1 cols 512:1024]
                    etp = pe_pool.tile([128, NKC, 2, 1024], bf16, tag="etp")
                    for kc in range(NKC):
                        ks = slice(kc * 128, (kc + 1) * 128)
                        for qh in range(2):
                            qsl = slice(q0 + qh * 512, q0 + (qh + 1) * 512)
                            ps = psS.tile([128, 1024], f32, tag="psS")
                            nc.tensor.matmul(
                                ps[:, 0:512],
                                kt_sb[0:64, hp, ks], qt_sb[0:64, hp, qsl],
                                start=True, stop=True, tile_position=(0, 0),
                            )
                            nc.tensor.matmul(
                                ps[:, 512:1024],
                                kt_sb[64:128, hp, ks], qt_sb[64:128, hp, qsl],
                                start=True, stop=True, tile_position=(64, 0),
                            )
                            # one exp per (k-chunk, q-half): both heads
                            nc.scalar.activation(
                                etp[:, kc, qh, :], ps[:], EXP, scale=SCALE)
                    # O'T = V'^T @ ET per head & q-half; row 64 = denominators
                    for h_i, h in enumerate((h0, h1)):
                        for qh in range(2):
                            po = psO.tile([HD + 1, 512], f32, tag="po")
                            esl = slice(h_i * 512, (h_i + 1) * 512)
                            for kc in range(NKC):
                                nc.tensor.matmul(
                                    po[:], vp_sb[:, kc, h, :],
                                    etp[:, kc, qh, esl],
                                    start=(kc == 0), stop=(kc == NKC - 1),
                                )
                            sums = prs.tile([1, 512], f32, tag="sums")
                            nc.vector.tensor_copy(sums[:], po[HD:HD + 1, :])
                            rf = prs.tile([1, 512], f32, tag="rf")
                            nc.vector.reciprocal_approx_fast(rf[:], sums[:])
                            rs = prs.tile([64, 512], f32, tag="rs")
                            nc.gpsimd.partition_broadcast(rs[:], rf[:])
                            prow = slice(h_i * 64, h_i * 64 + 64)
                            qsl = slice(q0 + qh * 512, q0 + (qh + 1) * 512)
                            nc.vector.tensor_mul(
                                attnT_sb[prow, hp, qsl], po[0:HD, :], rs[:])
                # output projection for this qt2's q columns (PE filler)
                for mo in range(qt2 * 8, qt2 * 8 + 8):
                    y_cols(mo)

        _pp_cm.__exit__(None, None, None)

    nc.finalize()
    return nc


def make_in_maps(x, context, Wq, Wk, Wv, Wo, bo):
    """Host-side sharding + layout prep: transpose and cast to bf16."""
    import ml_dtypes
    bf16 = ml_dtypes.bfloat16

    x = np.asarray(x, np.float32)
    context = np.asarray(context, np.float32)
    wq = np.asarray(Wq, np.float32).astype(bf16)
    wk = np.asarray(Wk, np.float32).astype(bf16)
    wv = np.asarray(Wv, np.float32).astype(bf16)
    wo = np.asarray(Wo, np.float32).astype(bf16)
    bo = np.asarray(bo, np.float32).reshape(1, QD).astype(bf16)

    in_maps = []
    for c in range(N_CORES):
        b, half = c // 2, c % 2
        xs = x[b, half * NQL:(half + 1) * NQL, :]           # [2048, 1024]
        in_maps.append({
            "xT": np.ascontiguousarray(xs.T).astype(bf16),   # [1024, 2048]
            "ctxT": np.ascontiguousarray(context[b].T).astype(bf16),  # [768, 1024]
            "wq": wq, "wk": wk, "wv": wv, "wo": wo, "bo": bo,
        })
    return in_maps


_NC_CACHE = {}


def kernel(x, context, Wq, Wk, Wv, Wo, bo, _trace=False):
    import sys
    if "/opt/trn_rl_repo" not in sys.path:
        sys.path.insert(0, "/opt/trn_rl_repo")
    from concourse.bass_utils import run_bass_kernel_spmd

    if "nc" not in _NC_CACHE:
        _NC_CACHE["nc"] = build_bass()
    nc = _NC_CACHE["nc"]

    in_maps = make_in_maps(x, context, Wq, Wk, Wv, Wo, bo)
    res = run_bass_kernel_spmd(
        nc, in_maps, core_ids=list(range(N_CORES)), trace=_trace)

    out = np.empty((B, NQ, QD), np.float32)
    for c in range(N_CORES):
        b, half = c // 2, c % 2
        out[b, half * NQL:(half + 1) * NQL, :] = res.results[c]["out"]
    if _trace:
        return out, res
    return out


# revision 29
# speedup vs baseline: 1.7587x; 1.0750x over previous
"""Cross-attention Trainium2 kernel, 8-core data-parallel.

Problem (hardcoded): B=4, NQ=4096, NK=1024, QD=1024, CD=768, H=16, HD=64.
  out = softmax((x@Wq) @ (ctx@Wk)^T / sqrt(HD)) @ (ctx@Wv) @ Wo + bo

Sharding: pure data-parallel. 8 cores = 4 batches x 2 NQ-halves of 2048
query rows. Each core redundantly computes K/V projections for its batch
(3.2+3.2 GF vs 18.8 GF total per core -- cheap) and needs no collectives.

Per-core layout strategy: every matmul keeps its contraction dim on SBUF
partitions, so the chain is computed fully "transposed":
  QT[qd,q]   = Wq^T-contract(xT)        (x pre-transposed on host)
  KT[qd,k]   = Wk^T-contract(ctxT)
  V'[k,qd+1] = ctx@Wv with a ones column appended per head
  ST[k,q]    = per head: KT_h^T-slices @ QT_h (row-tiled 2 heads/matmul)
  ET         = exp(ST * 1/8)            (ACT, scale folded in)
  O'T[65,q]  = V'_h^T @ ET_h            (row 64 = softmax denominators)
  attnT      = O'T[0:64] * recip(denoms) (DVE, recip broadcast via PE)
  out[q,od]  = attnT^T @ Wo + bo
"""

import numpy as np

B, NQ, NK = 4, 4096, 1024
QD, CD, H = 1024, 768, 16
HD = QD // H
SCALE = HD ** -0.5
NQL = NQ // 2          # query rows per core
N_CORES = 8


def build_bass():
    """Build the per-core Bass graph (SPMD, identical on all 8 cores)."""
    import concourse.bass as bass
    import concourse.tile as tile
    from concourse import bacc, mybir

    f32 = mybir.dt.float32
    bf16 = mybir.dt.bfloat16
    EXP = mybir.ActivationFunctionType.Exp

    nc = bacc.Bacc()

    xT_h = nc.dram_tensor("xT", (QD, NQL), bf16, kind="ExternalInput")
    ctxT_h = nc.dram_tensor("ctxT", (CD, NK), bf16, kind="ExternalInput")
    wq_h = nc.dram_tensor("wq", (QD, QD), bf16, kind="ExternalInput")
    wk_h = nc.dram_tensor("wk", (CD, QD), bf16, kind="ExternalInput")
    wv_h = nc.dram_tensor("wv", (CD, QD), bf16, kind="ExternalInput")
    wo_h = nc.dram_tensor("wo", (QD, QD), bf16, kind="ExternalInput")
    bo_h = nc.dram_tensor("bo", (1, QD), bf16, kind="ExternalInput")
    out_h = nc.dram_tensor("out", (NQL, QD), f32, kind="ExternalOutput")

    # DRAM views chunked to 128 partitions
    xT_d = xT_h[:].rearrange("(c p) n -> p c n", p=128)       # [128, 8, 2048]
    ctxT_d = ctxT_h[:].rearrange("(c p) n -> p c n", p=128)   # [128, 6, 1024]
    wq_d = wq_h[:].rearrange("(c p) m -> p c m", p=128)       # [128, 8, 1024]
    wk_d = wk_h[:].rearrange("(c p) m -> p c m", p=128)       # [128, 6, 1024]
    wv_d = wv_h[:].rearrange("(c p) m -> p c m", p=128)       # [128, 6, 1024]
    wo_d = wo_h[:].rearrange("(c p) m -> p c m", p=128)       # [128, 8, 1024]
    out_d = out_h[:].rearrange("(t p) n -> p t n", p=128)     # [128, 16, 1024]

    KC_Q = QD // 128   # 8  contraction chunks for Q/out projections
    KC_C = CD // 128   # 6  contraction chunks for K/V projections
    NKC = NK // 128    # 8  key chunks
    NQT = NQL // 512   # 4  query tiles of 512
    HP = H // 2        # 8  head pairs

    with tile.TileContext(nc) as tc:
        _pp_cm = tc.tile_pool(name="persist", bufs=1)
        pp = _pp_cm.__enter__()
        # ---- persistent tiles (live for the whole kernel)
        qt_sb = pp.tile([128, KC_Q, NQL], bf16)        # QT   4 MB
        kt_sb = pp.tile([128, KC_Q, NK], bf16)         # KT   2 MB
        vp_sb = pp.tile([128, NKC, H, HD + 1], bf16)   # V'   2.08 MB
        attnT_sb = pp.tile([128, KC_Q, NQL], bf16)     # attn 4 MB
        wo_sb = pp.tile([128, KC_Q, QD], bf16)         # Wo   2 MB
        bo_bc = pp.tile([128, QD], f32)                # bias broadcast
        ones_sb = pp.tile([1, 128], bf16)

        with (
            tc.tile_pool(name="inputs", bufs=1) as pin,
            tc.tile_pool(name="psA", bufs=2, space=bass.MemorySpace.PSUM) as psA,
        ):
            # ---- input tiles (freed after projections)
            xT_sb = pin.tile([128, KC_Q, NQL], bf16)       # 4 MB
            wq_sb = pin.tile([128, KC_Q, QD], bf16)        # 2 MB
            ctxT_sb = pin.tile([128, KC_C, NK], bf16)      # 1.5 MB
            wk_sb = pin.tile([128, KC_C, QD], bf16)
            wv_sb = pin.tile([128, KC_C, QD], bf16)
            bo_sb = pin.tile([1, QD], bf16)

            # stage DMA issues so each phase's inputs arrive just-in-time:
            # K needs ctxT+wk; V needs wv; Q needs xT+wq; Y needs wo.
            nc.sync.dma_start(ctxT_sb[:], ctxT_d)
            nc.sync.dma_start(wk_sb[:], wk_d)
            nc.sync.dma_start(bo_sb[:], bo_h[:])

            nc.vector.memset(ones_sb[:], 1.0)
            nc.vector.memset(vp_sb[:, :, :, HD], 1.0)      # ones column per head
            # broadcast bo across partitions via PE (ones outer product, bf16)
            for no in range(QD // 512):
                psb = psA.tile([128, 512], f32, tag="psA")
                nc.tensor.matmul(psb[:], ones_sb[:],
                                 bo_sb[0:1, no * 512:(no + 1) * 512],
                                 start=True, stop=True)
                nc.scalar.copy(bo_bc[:, no * 512:(no + 1) * 512], psb[:])

            # ---- phase K: KT[qd, k]
            for mo in range(KC_Q):
                for nk in range(NK // 512):
                    ps = psA.tile([128, 512], f32, tag="psA")
                    for c in range(KC_C):
                        nc.tensor.matmul(
                            ps[:],
                            wk_sb[:, c, mo * 128:(mo + 1) * 128],
                            ctxT_sb[:, c, nk * 512:(nk + 1) * 512],
                            start=(c == 0), stop=(c == KC_C - 1),
                        )
                    if (mo + nk) % 2 == 0:
                        nc.vector.tensor_copy(
                            kt_sb[:, mo, nk * 512:(nk + 1) * 512], ps[:])
                    else:
                        nc.scalar.copy(
                            kt_sb[:, mo, nk * 512:(nk + 1) * 512], ps[:])

            # ---- phase V: V[k, qd] (+ones col) strided into vp_sb
            nc.sync.dma_start(wv_sb[:], wv_d)
            nc.sync.dma_start(xT_sb[:], xT_d)
            nc.sync.dma_start(wq_sb[:], wq_d)
            for ko in range(NKC):
                for nv in range(QD // 512):
                    ps = psA.tile([128, 512], f32, tag="psA")
                    for c in range(KC_C):
                        nc.tensor.matmul(
                            ps[:],
                            ctxT_sb[:, c, ko * 128:(ko + 1) * 128],
                            wv_sb[:, c, nv * 512:(nv + 1) * 512],
                            start=(c == 0), stop=(c == KC_C - 1),
                        )
                    if (ko + nv) % 2 == 0:
                        nc.vector.tensor_copy(
                            vp_sb[:, ko, nv * 8:(nv + 1) * 8, 0:HD],
                            ps[:].rearrange("p (h d) -> p h d", h=8),
                        )
                    else:
                        nc.scalar.copy(
                            vp_sb[:, ko, nv * 8:(nv + 1) * 8, 0:HD],
                            ps[:].rearrange("p (h d) -> p h d", h=8),
                        )

            # ---- phase Q: QT[qd, q]
            nc.sync.dma_start(wo_sb[:], wo_d)
            for mo in range(KC_Q):
                for nq in range(NQT):
                    ps = psA.tile([128, 512], f32, tag="psA")
                    for c in range(KC_Q):
                        nc.tensor.matmul(
                            ps[:],
                            wq_sb[:, c, mo * 128:(mo + 1) * 128],
                            xT_sb[:, c, nq * 512:(nq + 1) * 512],
                            start=(c == 0), stop=(c == KC_Q - 1),
                        )
                    if (mo + nq) % 2 == 0:
                        nc.vector.tensor_copy(
                            qt_sb[:, mo, nq * 512:(nq + 1) * 512], ps[:])
                    else:
                        nc.scalar.copy(
                            qt_sb[:, mo, nq * 512:(nq + 1) * 512], ps[:])

        # ---- main loop: qt2-outer (1024-wide q tiles), head-pairs inner.
        # Wide F=2048 exp tiles (both heads share one 4-bank PSUM stage);
        # the output projection for each qt2's columns runs as PE filler
        # interleaved with the next qt2's attention.
        with (
            tc.tile_pool(name="et", bufs=2) as pe_pool,
            tc.tile_pool(name="rsmall", bufs=2) as prs,
            tc.tile_pool(name="yout", bufs=3) as py,
            tc.tile_pool(name="psS", bufs=2, space=bass.MemorySpace.PSUM) as psS,
            tc.tile_pool(name="psO", bufs=3, space=bass.MemorySpace.PSUM) as psO,
            tc.tile_pool(name="psX", bufs=1, space=bass.MemorySpace.PSUM) as psX,
        ):
            def y_cols(mo):
                # out rows [128mo : 128mo+128] = attnT^T @ Wo + bo
                for no in range(QD // 512):
                    ps = psX.tile([128, 512], f32, tag="psX")
                    for c in range(KC_Q):
                        nc.tensor.matmul(
                            ps[:],
                            attnT_sb[:, c, mo * 128:(mo + 1) * 128],
                            wo_sb[:, c, no * 512:(no + 1) * 512],
                            start=(c == 0), stop=(c == KC_Q - 1),
                        )
                    y = py.tile([128, 512], f32, tag="y")
                    nc.vector.tensor_add(
                        y[:], ps[:], bo_bc[:, no * 512:(no + 1) * 512])
                    nc.sync.dma_start(
                        out_d[:, mo, no * 512:(no + 1) * 512], y[:])

            for qt2 in range(2):
                for hp in range(HP):
                    h0, h1 = 2 * hp, 2 * hp + 1
                    q0 = qt2 * 1024
                    # ET layout: [128, kc, qh, h0 cols 0:512 | h1 cols 512:1024]
                    etp = pe_pool.tile([128, NKC, 2, 1024], bf16, tag="etp")
                    for kc in range(NKC):
                        ks = slice(kc * 128, (kc + 1) * 128)
                        for qh in range(2):
                            qsl = slice(q0 + qh * 512, q0 + (qh + 1) * 512)
                            ps = psS.tile([128, 1024], f32, tag="psS")
                            nc.tensor.matmul(
                                ps[:, 0:512],
                                kt_sb[0:64, hp, ks], qt_sb[0:64, hp, qsl],
                                start=True, stop=True, tile_position=(0, 0),
                            )
                            nc.tensor.matmul(
                                ps[:, 512:1024],
                                kt_sb[64:128, hp, ks], qt_sb[64:128, hp, qsl],
                                start=True, stop=True, tile_position=(64, 0),
                            )
                            # one exp per (k-chunk, q-half): both heads
                            nc.scalar.activation(
                                etp[:, kc, qh, :], ps[:], EXP, scale=SCALE)
                    # O'T = V'^T @ ET per head & q-half; row 64 = denominators
                    for h_i, h in enumerate((h0, h1)):
                        for qh in range(2):
                            po = psO.tile([HD + 1, 512], f32, tag="po")
                            esl = slice(h_i * 512, (h_i + 1) * 512)
                            for kc in range(NKC):
                                nc.tensor.matmul(
                                    po[:], vp_sb[:, kc, h, :],
                                    etp[:, kc, qh, esl],
                                    start=(kc == 0), stop=(kc == NKC - 1),
                                )
                            sums = prs.tile([1, 512], f32, tag="sums")
                            nc.vector.tensor_copy(sums[:], po[HD:HD + 1, :])
                            rf = prs.tile([1, 512], f32, tag="rf")
                            nc.vector.reciprocal_approx_fast(rf[:], sums[:])
                            rs = prs.tile([64, 512], f32, tag="rs")
                            nc.gpsimd.partition_broadcast(rs[:], rf[:])
                            prow = slice(h_i * 64, h_i * 64 + 64)
                            qsl = slice(q0 + qh * 512, q0 + (qh + 1) * 512)
                            nc.vector.tensor_mul(
                                attnT_sb[prow, hp, qsl], po[0:HD, :], rs[:])
                # output projection for this qt2's q columns (PE filler)
                for mo in range(qt2 * 8, qt2 * 8 + 8):
                    y_cols(mo)

        _pp_cm.__exit__(None, None, None)

    nc.finalize()
    return nc


def make_in_maps(x, context, Wq, Wk, Wv, Wo, bo):
    """Host-side sharding + layout prep: transpose and cast to bf16."""
    import ml_dtypes
    bf16 = ml_dtypes.bfloat16

    x = np.asarray(x, np.float32)
    context = np.asarray(context, np.float32)
    wq = np.asarray(Wq, np.float32).astype(bf16)
    wk = np.asarray(Wk, np.float32).astype(bf16)
    wv = np.asarray(Wv, np.float32).astype(bf16)
    wo = np.asarray(Wo, np.float32).astype(bf16)
    bo = np.asarray(bo, np.float32).reshape(1, QD).astype(bf16)

    in_maps = []
    for c in range(N_CORES):
        b, half = c // 2, c % 2
        xs = x[b, half * NQL:(half + 1) * NQL, :]           # [2048, 1024]
        in_maps.append({
            "xT": np.ascontiguousarray(xs.T).astype(bf16),   # [1024, 2048]
            "ctxT": np.ascontiguousarray(context[b].T).astype(bf16),  # [768, 1024]
            "wq": wq, "wk": wk, "wv": wv, "wo": wo, "bo": bo,
        })
    return in_maps


_NC_CACHE = {}


def kernel(x, context, Wq, Wk, Wv, Wo, bo, _trace=False):
    import sys
    if "/opt/trn_rl_repo" not in sys.path:
        sys.path.insert(0, "/opt/trn_rl_repo")
    from concourse.bass_utils import run_bass_kernel_spmd

    if "nc" not in _NC_CACHE:
        _NC_CACHE["nc"] = build_bass()
    nc = _NC_CACHE["nc"]

    in_maps = make_in_maps(x, context, Wq, Wk, Wv, Wo, bo)
    res = run_bass_kernel_spmd(
        nc, in_maps, core_ids=list(range(N_CORES)), trace=_trace)

    out = np.empty((B, NQ, QD), np.float32)
    for c in range(N_CORES):
        b, half = c // 2, c % 2
        out[b, half * NQL:(half + 1) * NQL, :] = res.results[c]["out"]
    if _trace:
        return out, res
    return out


# revision 30
# speedup vs baseline: 1.7661x; 1.0042x over previous
"""Cross-attention Trainium2 kernel, 8-core data-parallel.

Problem (hardcoded): B=4, NQ=4096, NK=1024, QD=1024, CD=768, H=16, HD=64.
  out = softmax((x@Wq) @ (ctx@Wk)^T / sqrt(HD)) @ (ctx@Wv) @ Wo + bo

Sharding: pure data-parallel. 8 cores = 4 batches x 2 NQ-halves of 2048
query rows. Each core redundantly computes K/V projections for its batch
(3.2+3.2 GF vs 18.8 GF total per core -- cheap) and needs no collectives.

Per-core layout strategy: every matmul keeps its contraction dim on SBUF
partitions, so the chain is computed fully "transposed":
  QT[qd,q]   = Wq^T-contract(xT)        (x pre-transposed on host)
  KT[qd,k]   = Wk^T-contract(ctxT)
  V'[k,qd+1] = ctx@Wv with a ones column appended per head
  ST[k,q]    = per head: KT_h^T-slices @ QT_h (row-tiled 2 heads/matmul)
  ET         = exp(ST * 1/8)            (ACT, scale folded in)
  O'T[65,q]  = V'_h^T @ ET_h            (row 64 = softmax denominators)
  attnT      = O'T[0:64] * recip(denoms) (DVE, recip broadcast via PE)
  out[q,od]  = attnT^T @ Wo + bo
"""

import numpy as np

B, NQ, NK = 4, 4096, 1024
QD, CD, H = 1024, 768, 16
HD = QD // H
SCALE = HD ** -0.5
NQL = NQ // 2          # query rows per core
N_CORES = 8


def build_bass():
    """Build the per-core Bass graph (SPMD, identical on all 8 cores)."""
    import concourse.bass as bass
    import concourse.tile as tile
    from concourse import bacc, mybir

    f32 = mybir.dt.float32
    bf16 = mybir.dt.bfloat16
    EXP = mybir.ActivationFunctionType.Exp

    nc = bacc.Bacc()

    xT_h = nc.dram_tensor("xT", (QD, NQL), bf16, kind="ExternalInput")
    ctxT_h = nc.dram_tensor("ctxT", (CD, NK), bf16, kind="ExternalInput")
    wq_h = nc.dram_tensor("wq", (QD, QD), bf16, kind="ExternalInput")
    wk_h = nc.dram_tensor("wk", (CD, QD), bf16, kind="ExternalInput")
    wv_h = nc.dram_tensor("wv", (CD, QD), bf16, kind="ExternalInput")
    wo_h = nc.dram_tensor("wo", (QD, QD), bf16, kind="ExternalInput")
    bo_h = nc.dram_tensor("bo", (1, QD), bf16, kind="ExternalInput")
    out_h = nc.dram_tensor("out", (NQL, QD), f32, kind="ExternalOutput")

    # DRAM views chunked to 128 partitions
    xT_d = xT_h[:].rearrange("(c p) n -> p c n", p=128)       # [128, 8, 2048]
    ctxT_d = ctxT_h[:].rearrange("(c p) n -> p c n", p=128)   # [128, 6, 1024]
    wq_d = wq_h[:].rearrange("(c p) m -> p c m", p=128)       # [128, 8, 1024]
    wk_d = wk_h[:].rearrange("(c p) m -> p c m", p=128)       # [128, 6, 1024]
    wv_d = wv_h[:].rearrange("(c p) m -> p c m", p=128)       # [128, 6, 1024]
    wo_d = wo_h[:].rearrange("(c p) m -> p c m", p=128)       # [128, 8, 1024]
    out_d = out_h[:].rearrange("(t p) n -> p t n", p=128)     # [128, 16, 1024]

    KC_Q = QD // 128   # 8  contraction chunks for Q/out projections
    KC_C = CD // 128   # 6  contraction chunks for K/V projections
    NKC = NK // 128    # 8  key chunks
    NQT = NQL // 512   # 4  query tiles of 512
    HP = H // 2        # 8  head pairs

    with tile.TileContext(nc) as tc:
        _pp_cm = tc.tile_pool(name="persist", bufs=1)
        pp = _pp_cm.__enter__()
        # ---- persistent tiles (live for the whole kernel)
        qt_sb = pp.tile([128, KC_Q, NQL], bf16)        # QT   4 MB
        kt_sb = pp.tile([128, KC_Q, NK], bf16)         # KT   2 MB
        vp_sb = pp.tile([128, NKC, H, HD + 1], bf16)   # V'   2.08 MB
        attnT_sb = pp.tile([128, KC_Q, NQL], bf16)     # attn 4 MB
        wo_sb = pp.tile([128, KC_Q, QD], bf16)         # Wo   2 MB
        bo_bc = pp.tile([128, QD], f32)                # bias broadcast
        ones_sb = pp.tile([1, 128], bf16)

        with (
            tc.tile_pool(name="inputs", bufs=1) as pin,
            tc.tile_pool(name="psA", bufs=2, space=bass.MemorySpace.PSUM) as psA,
        ):
            # ---- input tiles (freed after projections)
            xT_sb = pin.tile([128, KC_Q, NQL], bf16)       # 4 MB
            wq_sb = pin.tile([128, KC_Q, QD], bf16)        # 2 MB
            ctxT_sb = pin.tile([128, KC_C, NK], bf16)      # 1.5 MB
            wk_sb = pin.tile([128, KC_C, QD], bf16)
            wv_sb = pin.tile([128, KC_C, QD], bf16)
            bo_sb = pin.tile([1, QD], bf16)

            # stage DMA issues so each phase's inputs arrive just-in-time:
            # K needs ctxT+wk (split in halves so K(mo=0) starts sooner);
            # V needs wv; Q needs xT+wq; Y needs wo.
            nc.sync.dma_start(ctxT_sb[:], ctxT_d)
            nc.sync.dma_start(wk_sb[:, :, 0:512], wk_d[:, :, 0:512])
            nc.sync.dma_start(bo_sb[:], bo_h[:])
            nc.sync.dma_start(wk_sb[:, :, 512:1024], wk_d[:, :, 512:1024])

            nc.vector.memset(ones_sb[:], 1.0)
            nc.vector.memset(vp_sb[:, :, :, HD], 1.0)      # ones column per head
            # broadcast bo across partitions via PE (ones outer product, bf16)
            for no in range(QD // 512):
                psb = psA.tile([128, 512], f32, tag="psA")
                nc.tensor.matmul(psb[:], ones_sb[:],
                                 bo_sb[0:1, no * 512:(no + 1) * 512],
                                 start=True, stop=True)
                nc.scalar.copy(bo_bc[:, no * 512:(no + 1) * 512], psb[:])

            # ---- phase K: KT[qd, k]
            for mo in range(KC_Q):
                for nk in range(NK // 512):
                    ps = psA.tile([128, 512], f32, tag="psA")
                    for c in range(KC_C):
                        nc.tensor.matmul(
                            ps[:],
                            wk_sb[:, c, mo * 128:(mo + 1) * 128],
                            ctxT_sb[:, c, nk * 512:(nk + 1) * 512],
                            start=(c == 0), stop=(c == KC_C - 1),
                        )
                    if (mo + nk) % 2 == 0:
                        nc.vector.tensor_copy(
                            kt_sb[:, mo, nk * 512:(nk + 1) * 512], ps[:])
                    else:
                        nc.scalar.copy(
                            kt_sb[:, mo, nk * 512:(nk + 1) * 512], ps[:])

            # ---- phase V: V[k, qd] (+ones col) strided into vp_sb
            nc.sync.dma_start(wv_sb[:], wv_d)
            nc.sync.dma_start(xT_sb[:], xT_d)
            nc.sync.dma_start(wq_sb[:], wq_d)
            for ko in range(NKC):
                for nv in range(QD // 512):
                    ps = psA.tile([128, 512], f32, tag="psA")
                    for c in range(KC_C):
                        nc.tensor.matmul(
                            ps[:],
                            ctxT_sb[:, c, ko * 128:(ko + 1) * 128],
                            wv_sb[:, c, nv * 512:(nv + 1) * 512],
                            start=(c == 0), stop=(c == KC_C - 1),
                        )
                    if (ko + nv) % 2 == 0:
                        nc.vector.tensor_copy(
                            vp_sb[:, ko, nv * 8:(nv + 1) * 8, 0:HD],
                            ps[:].rearrange("p (h d) -> p h d", h=8),
                        )
                    else:
                        nc.scalar.copy(
                            vp_sb[:, ko, nv * 8:(nv + 1) * 8, 0:HD],
                            ps[:].rearrange("p (h d) -> p h d", h=8),
                        )

            # ---- phase Q: QT[qd, q]
            nc.sync.dma_start(wo_sb[:], wo_d)
            for mo in range(KC_Q):
                for nq in range(NQT):
                    ps = psA.tile([128, 512], f32, tag="psA")
                    for c in range(KC_Q):
                        nc.tensor.matmul(
                            ps[:],
                            wq_sb[:, c, mo * 128:(mo + 1) * 128],
                            xT_sb[:, c, nq * 512:(nq + 1) * 512],
                            start=(c == 0), stop=(c == KC_Q - 1),
                        )
                    if (mo + nq) % 2 == 0:
                        nc.vector.tensor_copy(
                            qt_sb[:, mo, nq * 512:(nq + 1) * 512], ps[:])
                    else:
                        nc.scalar.copy(
                            qt_sb[:, mo, nq * 512:(nq + 1) * 512], ps[:])

        # ---- main loop: qt2-outer (1024-wide q tiles), head-pairs inner.
        # Wide F=2048 exp tiles (both heads share one 4-bank PSUM stage);
        # the output projection for each qt2's columns runs as PE filler
        # interleaved with the next qt2's attention.
        with (
            tc.tile_pool(name="et", bufs=2) as pe_pool,
            tc.tile_pool(name="rsmall", bufs=2) as prs,
            tc.tile_pool(name="yout", bufs=3) as py,
            tc.tile_pool(name="psS", bufs=2, space=bass.MemorySpace.PSUM) as psS,
            tc.tile_pool(name="psO", bufs=3, space=bass.MemorySpace.PSUM) as psO,
            tc.tile_pool(name="psX", bufs=1, space=bass.MemorySpace.PSUM) as psX,
        ):
            def y_cols(mo):
                # out rows [128mo : 128mo+128] = attnT^T @ Wo + bo
                for no in range(QD // 512):
                    ps = psX.tile([128, 512], f32, tag="psX")
                    for c in range(KC_Q):
                        nc.tensor.matmul(
                            ps[:],
                            attnT_sb[:, c, mo * 128:(mo + 1) * 128],
                            wo_sb[:, c, no * 512:(no + 1) * 512],
                            start=(c == 0), stop=(c == KC_Q - 1),
                        )
                    y = py.tile([128, 512], f32, tag="y")
                    nc.vector.tensor_add(
                        y[:], ps[:], bo_bc[:, no * 512:(no + 1) * 512])
                    nc.sync.dma_start(
                        out_d[:, mo, no * 512:(no + 1) * 512], y[:])

            for qt2 in range(2):
                for hp in range(HP):
                    h0, h1 = 2 * hp, 2 * hp + 1
                    q0 = qt2 * 1024
                    # ET layout: [128, kc, qh, h0 cols 0:512 | h1 cols 512:1024]
                    etp = pe_pool.tile([128, NKC, 2, 1024], bf16, tag="etp")
                    for kc in range(NKC):
                        ks = slice(kc * 128, (kc + 1) * 128)
                        for qh in range(2):
                            qsl = slice(q0 + qh * 512, q0 + (qh + 1) * 512)
                            ps = psS.tile([128, 1024], f32, tag="psS")
                            nc.tensor.matmul(
                                ps[:, 0:512],
                                kt_sb[0:64, hp, ks], qt_sb[0:64, hp, qsl],
                                start=True, stop=True, tile_position=(0, 0),
                            )
                            nc.tensor.matmul(
                                ps[:, 512:1024],
                                kt_sb[64:128, hp, ks], qt_sb[64:128, hp, qsl],
                                start=True, stop=True, tile_position=(64, 0),
                            )
                            # one exp per (k-chunk, q-half): both heads
                            nc.scalar.activation(
                                etp[:, kc, qh, :], ps[:], EXP, scale=SCALE)
                    # O'T = V'^T @ ET per head & q-half; row 64 = denominators
                    for h_i, h in enumerate((h0, h1)):
                        for qh in range(2):
                            po = psO.tile([HD + 1, 512], f32, tag="po")
                            esl = slice(h_i * 512, (h_i + 1) * 512)
                            for kc in range(NKC):
                                nc.tensor.matmul(
                                    po[:], vp_sb[:, kc, h, :],
                                    etp[:, kc, qh, esl],
                                    start=(kc == 0), stop=(kc == NKC - 1),
                                )
                            sums = prs.tile([1, 512], f32, tag="sums")
                            nc.vector.tensor_copy(sums[:], po[HD:HD + 1, :])
                            rf = prs.tile([1, 512], f32, tag="rf")
                            nc.vector.reciprocal_approx_fast(rf[:], sums[:])
                            rs = prs.tile([64, 512], f32, tag="rs")
                            nc.gpsimd.partition_broadcast(rs[:], rf[:])
                            prow = slice(h_i * 64, h_i * 64 + 64)
                            qsl = slice(q0 + qh * 512, q0 + (qh + 1) * 512)
                            nc.vector.tensor_mul(
                                attnT_sb[prow, hp, qsl], po[0:HD, :], rs[:])
                # output projection for this qt2's q columns (PE filler)
                for mo in range(qt2 * 8, qt2 * 8 + 8):
                    y_cols(mo)

        _pp_cm.__exit__(None, None, None)

    nc.finalize()
    return nc


def make_in_maps(x, context, Wq, Wk, Wv, Wo, bo):
    """Host-side sharding + layout prep: transpose and cast to bf16."""
    import ml_dtypes
    bf16 = ml_dtypes.bfloat16

    x = np.asarray(x, np.float32)
    context = np.asarray(context, np.float32)
    wq = np.asarray(Wq, np.float32).astype(bf16)
    wk = np.asarray(Wk, np.float32).astype(bf16)
    wv = np.asarray(Wv, np.float32).astype(bf16)
    wo = np.asarray(Wo, np.float32).astype(bf16)
    bo = np.asarray(bo, np.float32).reshape(1, QD).astype(bf16)

    in_maps = []
    for c in range(N_CORES):
        b, half = c // 2, c % 2
        xs = x[b, half * NQL:(half + 1) * NQL, :]           # [2048, 1024]
        in_maps.append({
            "xT": np.ascontiguousarray(xs.T).astype(bf16),   # [1024, 2048]
            "ctxT": np.ascontiguousarray(context[b].T).astype(bf16),  # [768, 1024]
            "wq": wq, "wk": wk, "wv": wv, "wo": wo, "bo": bo,
        })
    return in_maps


_NC_CACHE = {}


def kernel(x, context, Wq, Wk, Wv, Wo, bo, _trace=False):
    import sys
    if "/opt/trn_rl_repo" not in sys.path:
        sys.path.insert(0, "/opt/trn_rl_repo")
    from concourse.bass_utils import run_bass_kernel_spmd

    if "nc" not in _NC_CACHE:
        _NC_CACHE["nc"] = build_bass()
    nc = _NC_CACHE["nc"]

    in_maps = make_in_maps(x, context, Wq, Wk, Wv, Wo, bo)
    res = run_bass_kernel_spmd(
        nc, in_maps, core_ids=list(range(N_CORES)), trace=_trace)

    out = np.empty((B, NQ, QD), np.float32)
    for c in range(N_CORES):
        b, half = c // 2, c % 2
        out[b, half * NQL:(half + 1) * NQL, :] = res.results[c]["out"]
    if _trace:
        return out, res
    return out


# revision 32
# speedup vs baseline: 1.9564x; 1.1077x over previous
"""Cross-attention Trainium2 kernel, 8-core data-parallel.

Problem (hardcoded): B=4, NQ=4096, NK=1024, QD=1024, CD=768, H=16, HD=64.
  out = softmax((x@Wq) @ (ctx@Wk)^T / sqrt(HD)) @ (ctx@Wv) @ Wo + bo

Sharding: pure data-parallel. 8 cores = 4 batches x 2 NQ-halves of 2048
query rows. Each core redundantly computes K/V projections for its batch
(3.2+3.2 GF vs 18.8 GF total per core -- cheap) and needs no collectives.

Per-core layout strategy: every matmul keeps its contraction dim on SBUF
partitions, so the chain is computed fully "transposed":
  QT[qd,q]   = Wq^T-contract(xT)        (x pre-transposed on host)
  KT[qd,k]   = Wk^T-contract(ctxT)
  V'[k,qd+1] = ctx@Wv with a ones column appended per head
  ST[k,q]    = per head: KT_h^T-slices @ QT_h (row-tiled 2 heads/matmul)
  ET         = exp(ST * 1/8)            (ACT, scale folded in)
  O'T[65,q]  = V'_h^T @ ET_h            (row 64 = softmax denominators)
  attnT      = O'T[0:64] * recip(denoms) (DVE, recip broadcast via PE)
  out[q,od]  = attnT^T @ Wo + bo
"""

import numpy as np

B, NQ, NK = 4, 4096, 1024
QD, CD, H = 1024, 768, 16
HD = QD // H
SCALE = HD ** -0.5
NQL = NQ // 2          # query rows per core
N_CORES = 8


def build_bass():
    """Build the per-core Bass graph (SPMD, identical on all 8 cores)."""
    import concourse.bass as bass
    import concourse.tile as tile
    from concourse import bacc, mybir

    f32 = mybir.dt.float32
    bf16 = mybir.dt.bfloat16
    EXP = mybir.ActivationFunctionType.Exp

    nc = bacc.Bacc()

    xT_h = nc.dram_tensor("xT", (QD, NQL), bf16, kind="ExternalInput")
    ctxT_h = nc.dram_tensor("ctxT", (CD, NK), bf16, kind="ExternalInput")
    wq_h = nc.dram_tensor("wq", (QD, QD), bf16, kind="ExternalInput")
    wk_h = nc.dram_tensor("wk", (CD, QD), bf16, kind="ExternalInput")
    wv_h = nc.dram_tensor("wv", (CD, QD), bf16, kind="ExternalInput")
    wo_h = nc.dram_tensor("wo", (QD, QD), bf16, kind="ExternalInput")
    bo_h = nc.dram_tensor("bo", (1, QD), bf16, kind="ExternalInput")
    out_h = nc.dram_tensor("out", (NQL, QD), f32, kind="ExternalOutput")

    # DRAM views chunked to 128 partitions
    xT_d = xT_h[:].rearrange("(c p) n -> p c n", p=128)       # [128, 8, 2048]
    ctxT_d = ctxT_h[:].rearrange("(c p) n -> p c n", p=128)   # [128, 6, 1024]
    wq_d = wq_h[:].rearrange("(c p) m -> p c m", p=128)       # [128, 8, 1024]
    wk_d = wk_h[:].rearrange("(c p) m -> p c m", p=128)       # [128, 6, 1024]
    wv_d = wv_h[:].rearrange("(c p) m -> p c m", p=128)       # [128, 6, 1024]
    wo_d = wo_h[:].rearrange("(c p) m -> p c m", p=128)       # [128, 8, 1024]
    out_d = out_h[:].rearrange("(t p) n -> p t n", p=128)     # [128, 16, 1024]

    KC_Q = QD // 128   # 8  contraction chunks for Q/out projections
    KC_C = CD // 128   # 6  contraction chunks for K/V projections
    NKC = NK // 128    # 8  key chunks
    NQT = NQL // 512   # 4  query tiles of 512
    HP = H // 2        # 8  head pairs

    with tile.TileContext(nc) as tc:
        _pp_cm = tc.tile_pool(name="persist", bufs=1)
        pp = _pp_cm.__enter__()
        # ---- persistent tiles (live for the whole kernel)
        qt_sb = pp.tile([128, KC_Q, NQL], bf16)        # QT   4 MB
        kt_sb = pp.tile([128, KC_Q, NK], bf16)         # KT   2 MB
        vp_sb = pp.tile([128, NKC, H, HD + 1], bf16)   # V'   2.08 MB
        attnT_sb = pp.tile([128, KC_Q, NQL], bf16)     # attn 4 MB
        wo_sb = pp.tile([128, KC_Q, QD], bf16)         # Wo   2 MB
        bo_bc = pp.tile([128, QD], f32)                # bias broadcast
        ones_sb = pp.tile([1, 128], bf16)

        with (
            tc.tile_pool(name="inputs", bufs=1) as pin,
            tc.tile_pool(name="psA", bufs=2, space=bass.MemorySpace.PSUM) as psA,
        ):
            # ---- input tiles (freed after projections)
            xT_sb = pin.tile([128, KC_Q, NQL], bf16)       # 4 MB
            wq_sb = pin.tile([128, KC_Q, QD], bf16)        # 2 MB
            ctxT_sb = pin.tile([128, KC_C, NK], bf16)      # 1.5 MB
            wk_sb = pin.tile([128, KC_C, QD], bf16)
            wv_sb = pin.tile([128, KC_C, QD], bf16)
            bo_sb = pin.tile([1, QD], bf16)

            # stage DMA issues so each phase's inputs arrive just-in-time:
            # K needs ctxT+wk (split in halves so K(mo=0) starts sooner);
            # V needs wv; Q needs xT+wq; Y needs wo.
            nc.sync.dma_start(ctxT_sb[:, :, 0:512], ctxT_d[:, :, 0:512])
            nc.sync.dma_start(wk_sb[:, :, 0:256], wk_d[:, :, 0:256])
            nc.sync.dma_start(bo_sb[:], bo_h[:])
            nc.sync.dma_start(ctxT_sb[:, :, 512:1024], ctxT_d[:, :, 512:1024])
            nc.sync.dma_start(wk_sb[:, :, 256:1024], wk_d[:, :, 256:1024])

            nc.vector.memset(ones_sb[:], 1.0)
            nc.vector.memset(vp_sb[:, :, :, HD], 1.0)      # ones column per head
            # broadcast bo across partitions via PE (ones outer product, bf16)
            for no in range(QD // 512):
                psb = psA.tile([128, 512], f32, tag="psA")
                nc.tensor.matmul(psb[:], ones_sb[:],
                                 bo_sb[0:1, no * 512:(no + 1) * 512],
                                 start=True, stop=True)
                nc.scalar.copy(bo_bc[:, no * 512:(no + 1) * 512], psb[:])

            # ---- phase K: KT[qd, k]
            for mo in range(KC_Q):
                for nk in range(NK // 512):
                    ps = psA.tile([128, 512], f32, tag="psA")
                    for c in range(KC_C):
                        nc.tensor.matmul(
                            ps[:],
                            wk_sb[:, c, mo * 128:(mo + 1) * 128],
                            ctxT_sb[:, c, nk * 512:(nk + 1) * 512],
                            start=(c == 0), stop=(c == KC_C - 1),
                        )
                    if (mo + nk) % 2 == 0:
                        nc.vector.tensor_copy(
                            kt_sb[:, mo, nk * 512:(nk + 1) * 512], ps[:])
                    else:
                        nc.scalar.copy(
                            kt_sb[:, mo, nk * 512:(nk + 1) * 512], ps[:])

            # ---- phase V: V[k, qd] (+ones col) strided into vp_sb
            nc.sync.dma_start(wv_sb[:], wv_d)
            nc.sync.dma_start(xT_sb[:], xT_d)
            nc.sync.dma_start(wq_sb[:], wq_d)
            for ko in range(NKC):
                for nv in range(QD // 512):
                    ps = psA.tile([128, 512], f32, tag="psA")
                    for c in range(KC_C):
                        nc.tensor.matmul(
                            ps[:],
                            ctxT_sb[:, c, ko * 128:(ko + 1) * 128],
                            wv_sb[:, c, nv * 512:(nv + 1) * 512],
                            start=(c == 0), stop=(c == KC_C - 1),
                        )
                    if (ko + nv) % 2 == 0:
                        nc.vector.tensor_copy(
                            vp_sb[:, ko, nv * 8:(nv + 1) * 8, 0:HD],
                            ps[:].rearrange("p (h d) -> p h d", h=8),
                        )
                    else:
                        nc.scalar.copy(
                            vp_sb[:, ko, nv * 8:(nv + 1) * 8, 0:HD],
                            ps[:].rearrange("p (h d) -> p h d", h=8),
                        )

            # ---- phase Q: QT[qd, q]
            nc.sync.dma_start(wo_sb[:], wo_d)
            for mo in range(KC_Q):
                for nq in range(NQT):
                    ps = psA.tile([128, 512], f32, tag="psA")
                    for c in range(KC_Q):
                        nc.tensor.matmul(
                            ps[:],
                            wq_sb[:, c, mo * 128:(mo + 1) * 128],
                            xT_sb[:, c, nq * 512:(nq + 1) * 512],
                            start=(c == 0), stop=(c == KC_Q - 1),
                        )
                    if (mo + nq) % 2 == 0:
                        nc.vector.tensor_copy(
                            qt_sb[:, mo, nq * 512:(nq + 1) * 512], ps[:])
                    else:
                        nc.scalar.copy(
                            qt_sb[:, mo, nq * 512:(nq + 1) * 512], ps[:])

        # ---- main loop: qt2-outer (1024-wide q tiles), head-pairs inner.
        # Wide F=2048 exp tiles (both heads share one 4-bank PSUM stage);
        # the output projection for each qt2's columns runs as PE filler
        # interleaved with the next qt2's attention.
        with (
            tc.tile_pool(name="et", bufs=2) as pe_pool,
            tc.tile_pool(name="rsmall", bufs=2) as prs,
            tc.tile_pool(name="yout", bufs=3) as py,
            tc.tile_pool(name="psS", bufs=2, space=bass.MemorySpace.PSUM) as psS,
            tc.tile_pool(name="psO", bufs=3, space=bass.MemorySpace.PSUM) as psO,
            tc.tile_pool(name="psX", bufs=1, space=bass.MemorySpace.PSUM) as psX,
        ):
            def y_cols(mo):
                # out rows [128mo : 128mo+128] = attnT^T @ Wo + bo
                for no in range(QD // 512):
                    ps = psX.tile([128, 512], f32, tag="psX")
                    for c in range(KC_Q):
                        nc.tensor.matmul(
                            ps[:],
                            attnT_sb[:, c, mo * 128:(mo + 1) * 128],
                            wo_sb[:, c, no * 512:(no + 1) * 512],
                            start=(c == 0), stop=(c == KC_Q - 1),
                        )
                    y = py.tile([128, 512], f32, tag="y")
                    nc.vector.tensor_add(
                        y[:], ps[:], bo_bc[:, no * 512:(no + 1) * 512])
                    nc.sync.dma_start(
                        out_d[:, mo, no * 512:(no + 1) * 512], y[:])

            for qt2 in range(2):
                for hp in range(HP):
                    h0, h1 = 2 * hp, 2 * hp + 1
                    q0 = qt2 * 1024
                    # ET layout: [128, kc, qh, h0 cols 0:512 | h1 cols 512:1024]
                    # qh-outer: attn@V for q-half 0 overlaps q-half 1's exps
                    etp = pe_pool.tile([128, NKC, 2, 1024], bf16, tag="etp")
                    for qh in range(2):
                        qsl = slice(q0 + qh * 512, q0 + (qh + 1) * 512)
                        for kc in range(NKC):
                            ks = slice(kc * 128, (kc + 1) * 128)
                            ps = psS.tile([128, 1024], f32, tag="psS")
                            nc.tensor.matmul(
                                ps[:, 0:512],
                                kt_sb[0:64, hp, ks], qt_sb[0:64, hp, qsl],
                                start=True, stop=True, tile_position=(0, 0),
                            )
                            nc.tensor.matmul(
                                ps[:, 512:1024],
                                kt_sb[64:128, hp, ks], qt_sb[64:128, hp, qsl],
                                start=True, stop=True, tile_position=(64, 0),
                            )
                            # one exp per (k-chunk, q-half): both heads
                            nc.scalar.activation(
                                etp[:, kc, qh, :], ps[:], EXP, scale=SCALE)
                        # O'T = V'^T @ ET per head; row 64 = denominators
                        for h_i, h in enumerate((h0, h1)):
                            po = psO.tile([HD + 1, 512], f32, tag="po")
                            esl = slice(h_i * 512, (h_i + 1) * 512)
                            for kc in range(NKC):
                                nc.tensor.matmul(
                                    po[:], vp_sb[:, kc, h, :],
                                    etp[:, kc, qh, esl],
                                    start=(kc == 0), stop=(kc == NKC - 1),
                                )
                            sums = prs.tile([1, 512], f32, tag="sums")
                            nc.vector.tensor_copy(sums[:], po[HD:HD + 1, :])
                            rf = prs.tile([1, 512], f32, tag="rf")
                            nc.vector.reciprocal_approx_fast(rf[:], sums[:])
                            rs = prs.tile([64, 512], f32, tag="rs")
                            nc.gpsimd.partition_broadcast(rs[:], rf[:])
                            prow = slice(h_i * 64, h_i * 64 + 64)
                            nc.vector.tensor_mul(
                                attnT_sb[prow, hp, qsl], po[0:HD, :], rs[:])
                # output projection for this qt2's q columns (PE filler)
                for mo in range(qt2 * 8, qt2 * 8 + 8):
                    y_cols(mo)

        _pp_cm.__exit__(None, None, None)

    nc.finalize()
    return nc


def make_in_maps(x, context, Wq, Wk, Wv, Wo, bo):
    """Host-side sharding + layout prep: transpose and cast to bf16."""
    import ml_dtypes
    bf16 = ml_dtypes.bfloat16

    x = np.asarray(x, np.float32)
    context = np.asarray(context, np.float32)
    wq = np.asarray(Wq, np.float32).astype(bf16)
    wk = np.asarray(Wk, np.float32).astype(bf16)
    wv = np.asarray(Wv, np.float32).astype(bf16)
    wo = np.asarray(Wo, np.float32).astype(bf16)
    bo = np.asarray(bo, np.float32).reshape(1, QD).astype(bf16)

    in_maps = []
    for c in range(N_CORES):
        b, half = c // 2, c % 2
        xs = x[b, half * NQL:(half + 1) * NQL, :]           # [2048, 1024]
        in_maps.append({
            "xT": np.ascontiguousarray(xs.T).astype(bf16),   # [1024, 2048]
            "ctxT": np.ascontiguousarray(context[b].T).astype(bf16),  # [768, 1024]
            "wq": wq, "wk": wk, "wv": wv, "wo": wo, "bo": bo,
        })
    return in_maps


_NC_CACHE = {}


def kernel(x, context, Wq, Wk, Wv, Wo, bo, _trace=False):
    import sys
    if "/opt/trn_rl_repo" not in sys.path:
        sys.path.insert(0, "/opt/trn_rl_repo")
    from concourse.bass_utils import run_bass_kernel_spmd

    if "nc" not in _NC_CACHE:
        _NC_CACHE["nc"] = build_bass()
    nc = _NC_CACHE["nc"]

    in_maps = make_in_maps(x, context, Wq, Wk, Wv, Wo, bo)
    res = run_bass_kernel_spmd(
        nc, in_maps, core_ids=list(range(N_CORES)), trace=_trace)

    out = np.empty((B, NQ, QD), np.float32)
    for c in range(N_CORES):
        b, half = c // 2, c % 2
        out[b, half * NQL:(half + 1) * NQL, :] = res.results[c]["out"]
    if _trace:
        return out, res
    return out
